# revision 23
# baseline (speedup 1.0000x reference)
"""3-layer GAT (50k nodes, 1.6M edges) on 8 Trainium2 NeuronCores — v3.

v2 edge-parallel-by-destination strategy, with the engine-level hotspots
rebalanced based on the NTFF profile (Vector 85% busy, GpSimd ~100% busy
during aggregation, Scalar 3%):
  - gathers rotate across 4 SWDGE queues (was 1).
  - one-hot build: ONE batched is_equal per chunk for all blocks.
  - pair transposes: 4 per PSUM tile, drained by the Scalar engine.
  - alde matmuls col-sliced into ONE PSUM tile per chunk, one scalar drain.
  - single gather tile per chunk (A+B) -> single zt/lrelu/exp/msg ops.
  - table rows are [h | als] (no ones columns); the per-edge weight lands
    in the message tile via a Scalar-engine copy instead.
  - PSUM->SBUF drains (eacc, eluT, alde, ohT, table rows) moved to the
    Scalar engine; DVE keeps only the arithmetic.
"""

import numpy as np
import ml_dtypes

P = 128
N_NODES = 50000
N_EDGES = 1600000
F_IN = 128
N_CORES = 8
RNODES = N_NODES // N_CORES          # 6250 nodes per core
W = 64                               # scatter window (nodes)
NWIN = (RNODES + W - 1) // W         # 98 windows per core
BCHUNK = (RNODES + P - 1) // P       # 49 table-build chunks per region
HALF = 25000                         # gather table half size (int16 limit)
CHUNK_BLOCKS = 48                    # max 128-edge blocks per aggregation chunk
NQ = 4                               # SWDGE queues for gathers
SPLITW = 66                          # eluT AllGather split point (windows)
SPLITC = SPLITW * W                  # ... in eluT columns

# per-layer (H, C, ELEM): ELEM = bf16 elements per table row (256B aligned)
# row layout: [h (H*C) | als (H) | pad]
LAYERS = [
    dict(H=4, C=16, FIN=128, ELEM=128),
    dict(H=4, C=16, FIN=64, ELEM=128),
    dict(H=6, C=40, FIN=64, ELEM=256),
]

BF16 = ml_dtypes.bfloat16


def _preprocess(edge_index):
    """Sort edges by dst, split per core / window / table-half, pad each
    (window, half) group to a uniform (across cores) multiple of 128."""
    src = np.concatenate([edge_index[0], np.arange(N_NODES, dtype=np.int64)])
    dst = np.concatenate([edge_index[1], np.arange(N_NODES, dtype=np.int64)])
    order = np.argsort(dst, kind="stable")
    src = src[order].astype(np.int64)
    dst = dst[order].astype(np.int64)

    nwh = N_CORES * NWIN
    counts = np.zeros((N_CORES, NWIN, 2), np.int64)
    groups = [[None, None] for _ in range(nwh)]
    for d in range(N_CORES):
        lo_d = d * RNODES
        for w in range(NWIN):
            a = lo_d + w * W
            b = min(lo_d + (w + 1) * W, lo_d + RNODES)
            i0 = np.searchsorted(dst, a)
            i1 = np.searchsorted(dst, b)
            s = src[i0:i1]
            t = dst[i0:i1]
            selA = s < HALF
            groups[d * NWIN + w][0] = (s[selA], t[selA] - a)
            groups[d * NWIN + w][1] = (s[~selA] - HALF, t[~selA] - a)
            counts[d, w, 0] = int(selA.sum())
            counts[d, w, 1] = int((~selA).sum())

    mA = np.ceil(counts[:, :, 0].max(axis=0) / P).astype(int)   # [NWIN]
    mB = np.ceil(counts[:, :, 1].max(axis=0) / P).astype(int)

    chunks = []
    cur, cur_blocks = [], 0
    for w in range(NWIN):
        blk = int(mA[w] + mB[w])
        assert blk <= CHUNK_BLOCKS, f"window {w} too big: {blk} blocks"
        if cur_blocks + blk > CHUNK_BLOCKS:
            chunks.append(cur)
            cur, cur_blocks = [], 0
        cur.append(w)
        cur_blocks += blk
    if cur:
        chunks.append(cur)

    totA = int(mA.sum()) * P
    totB = int(mB.sum()) * P
    nblk_tot = int(mA.sum() + mB.sum())

    meta = dict(mA=mA, mB=mB, chunks=chunks, totA=totA, totB=totB,
                nblk_tot=nblk_tot)

    percore = []
    for d in range(N_CORES):
        eA = np.zeros(totA, np.int16)
        eB = np.zeros(totB, np.int16)
        dstcol = np.full(nblk_tot * P, 99.0, BF16)
        pa = pb = 0
        gb = 0
        for ws in chunks:
            for w in ws:
                s, tl = groups[d * NWIN + w][0]
                n = len(s)
                eA[pa:pa + n] = s
                dstcol[gb * P: gb * P + n] = tl.astype(BF16)
                pa += mA[w] * P
                gb += int(mA[w])
            for w in ws:
                s, tl = groups[d * NWIN + w][1]
                n = len(s)
                eB[pb:pb + n] = s
                dstcol[gb * P: gb * P + n] = tl.astype(BF16)
                pb += mB[w] * P
                gb += int(mB[w])
        idxA = np.tile(eA.reshape(-1, 16).T, (8, 1)) if totA else np.zeros((128, 0), np.int16)
        idxB = np.tile(eB.reshape(-1, 16).T, (8, 1)) if totB else np.zeros((128, 0), np.int16)
        percore.append(dict(
            idxA=np.ascontiguousarray(idxA),
            idxB=np.ascontiguousarray(idxB),
            dstcol=np.ascontiguousarray(dstcol.reshape(nblk_tot, P).T),
        ))
    return meta, percore


def _block_table(meta):
    """Per chunk: block layout [A-blocks by window | B-blocks by window]."""
    mA, mB, chunks = meta["mA"], meta["mB"], meta["chunks"]
    out = []
    gb = 0
    offA = offB = 0
    for ws in chunks:
        nAblk = int(sum(mA[w] for w in ws))
        nBblk = int(sum(mB[w] for w in ws))
        blocks = []   # (w, half, local_block_in_chunk, global_block)
        lb = 0
        for w in ws:
            for _ in range(int(mA[w])):
                blocks.append((w, 0, lb, gb)); lb += 1; gb += 1
        for w in ws:
            for _ in range(int(mB[w])):
                blocks.append((w, 1, lb, gb)); lb += 1; gb += 1
        # pairs within each (window, half) run; lone pair at odd run ends
        pairs = []   # (local_block_of_first, window)
        i = 0
        nblk = len(blocks)
        while i < nblk:
            w_i, half_i = blocks[i][0], blocks[i][1]
            j = i
            while j < nblk and blocks[j][0] == w_i and blocks[j][1] == half_i:
                j += 1
            for k in range(i, j, 2):
                pairs.append((k, w_i))
            i = j
        out.append(dict(ws=ws, nAblk=nAblk, nBblk=nBblk, blocks=blocks,
                        offA=offA, offB=offB, pairs=pairs))
        offA += nAblk * P
        offB += nBblk * P
    return out


def _prep_weights(inputs):
    """Host-side constant prep: combined [W | W@As | W@Ad] per layer."""
    x = np.asarray(inputs["x"], np.float32)
    d = {}
    d["xT"] = np.ascontiguousarray(x.T).astype(BF16)            # [128, 50000]
    for i, (wk, ask, adk, bk) in enumerate(
            [("W1", "a1s", "a1d", "b1"), ("W2", "a2s", "a2d", "b2"),
             ("W3", "a3s", "a3d", "b3")]):
        L = LAYERS[i]
        H, C = L["H"], L["C"]
        Wm = np.asarray(inputs[wk], np.float32)                  # [FIN, H*C]
        a_s = np.asarray(inputs[ask], np.float32)                # [H, C]
        a_d = np.asarray(inputs[adk], np.float32)
        b = np.asarray(inputs[bk], np.float32)
        # als[n,h] = sum_c h[n,h,c]*a_s[h,c] = h @ Asm, Asm[h*C+c, h]=a_s[h,c]
        Asm = np.zeros((H * C, H), np.float32)
        Adm = np.zeros((H * C, H), np.float32)
        for h in range(H):
            Asm[h * C:(h + 1) * C, h] = a_s[h]
            Adm[h * C:(h + 1) * C, h] = a_d[h]
        CW = np.concatenate([Wm, Wm @ Asm, Wm @ Adm], axis=1)    # [FIN, HC+2H]
        d[f"CW{i}"] = CW.astype(BF16)
        d[f"b{i}"] = np.tile(b.reshape(1, -1), (P, 1)).astype(np.float32)
    return d


def build_program(meta, n_cores=None):
    import concourse.bacc as bacc
    import concourse.bass as bass
    import concourse.tile as tile
    import concourse.mybir as mybir
    from concourse.masks import make_identity

    dt = mybir.dt
    AF = mybir.ActivationFunctionType
    OP = mybir.AluOpType

    if n_cores is None:
        n_cores = N_CORES
    mA, mB, chunks = meta["mA"], meta["mB"], meta["chunks"]
    totA, totB, nblk_tot = meta["totA"], meta["totB"], meta["nblk_tot"]
    btab = _block_table(meta)
    max_npair = max(len(cb["pairs"]) for cb in btab)

    nc = bacc.Bacc("TRN2", target_bir_lowering=False, debug=False,
                   num_swdge_queues=NQ)

    # ---------------- I/O ----------------
    t_xT = nc.dram_tensor("xT", [P, N_NODES], dt.bfloat16, kind="ExternalInput")
    t_CW = [nc.dram_tensor(f"CW{i}", [LAYERS[i]["FIN"],
                                      LAYERS[i]["H"] * LAYERS[i]["C"] + 2 * LAYERS[i]["H"]],
                           dt.bfloat16, kind="ExternalInput") for i in range(3)]
    t_b = [nc.dram_tensor(f"b{i}", [P, LAYERS[i]["H"] * LAYERS[i]["C"] if i < 2 else LAYERS[i]["C"]],
                          dt.float32, kind="ExternalInput") for i in range(3)]
    t_sel = nc.dram_tensor("sel", [P, N_CORES], dt.float32, kind="ExternalInput")
    t_idxA = nc.dram_tensor("idxA", [P, max(totA // 16, 1)], dt.int16, kind="ExternalInput")
    t_idxB = nc.dram_tensor("idxB", [P, max(totB // 16, 1)], dt.int16, kind="ExternalInput")
    t_dstcol = nc.dram_tensor("dstcol", [P, nblk_tot], dt.bfloat16, kind="ExternalInput")
    t_out = nc.dram_tensor("out", [RNODES, LAYERS[2]["C"]], dt.float32, kind="ExternalOutput")

    # ---------------- internal DRAM ----------------
    t_tab12 = nc.dram_tensor("tab12", [N_NODES, 128], dt.bfloat16)
    t_tab3 = nc.dram_tensor("tab3", [N_NODES, 256], dt.bfloat16)
    # eluT AllGather split into two contiguous buffers so the first
    # collective can fire mid-aggregation (collectives reject column slices)
    NCOLS2 = BCHUNK * P - SPLITC
    t_ccinA = [nc.dram_tensor(f"ccinA{i}", [64, SPLITC], dt.bfloat16) for i in range(2)]
    t_ccinB = [nc.dram_tensor(f"ccinB{i}", [64, NCOLS2], dt.bfloat16) for i in range(2)]
    t_agA = [nc.dram_tensor(f"agA{i}", [N_CORES, 64, SPLITC], dt.bfloat16,
                            addr_space="Shared") for i in range(2)]
    t_agB = [nc.dram_tensor(f"agB{i}", [N_CORES, 64, NCOLS2], dt.bfloat16,
                            addr_space="Shared") for i in range(2)]

    core_ids = list(range(n_cores))
    qc = [0]   # gather queue rotation counter

    with tile.TileContext(nc) as tc:
        with tc.tile_pool(name="const", bufs=1) as cpool, \
             tc.tile_pool(name="persist", bufs=1) as ppool, \
             tc.tile_pool(name="build", bufs=3) as bpool, \
             tc.tile_pool(name="gath", bufs=2) as gpool, \
             tc.tile_pool(name="edge", bufs=2) as epool, \
             tc.tile_pool(name="blk", bufs=6) as kpool, \
             tc.tile_pool(name="win", bufs=2) as wpool, \
             tc.tile_pool(name="bps", bufs=2, space="PSUM") as bps, \
             tc.tile_pool(name="aps", bufs=2, space="PSUM") as aps:

            # ----- constants -----
            ident = cpool.tile([P, P], dt.bfloat16, tag="ident")
            make_identity(nc, ident[:])
            iota_i = cpool.tile([P, W], dt.int32, tag="iotai")
            nc.gpsimd.iota(iota_i[:], [[1, W]], base=0, channel_multiplier=0)
            iota_row = cpool.tile([P, W], dt.bfloat16, tag="iotarow")
            nc.vector.tensor_copy(iota_row[:], iota_i[:])
            sel = cpool.tile([P, N_CORES], dt.float32, tag="sel")
            nc.sync.dma_start(sel[:], t_sel[:])
            CW_sb, b_sb = [], []
            for i in range(3):
                L = LAYERS[i]
                HC2H = L["H"] * L["C"] + 2 * L["H"]
                wt = cpool.tile([L["FIN"], HC2H], dt.bfloat16, tag=f"CW{i}")
                nc.sync.dma_start(wt[:], t_CW[i][:])
                CW_sb.append(wt)
                bt = cpool.tile([P, L["H"] * L["C"] if i < 2 else L["C"]], dt.float32, tag=f"b{i}")
                nc.sync.dma_start(bt[:], t_b[i][:])
                b_sb.append(bt)

            # ----- resident edge data -----
            idxA_sb = ppool.tile([P, max(totA // 16, 1)], dt.int16, tag="idxA")
            nc.sync.dma_start(idxA_sb[:], t_idxA[:])
            idxB_sb = ppool.tile([P, max(totB // 16, 1)], dt.int16, tag="idxB")
            nc.sync.dma_start(idxB_sb[:], t_idxB[:])
            dstcol_sb = ppool.tile([P, nblk_tot], dt.bfloat16, tag="dstcol")
            nc.sync.dma_start(dstcol_sb[:], t_dstcol[:])

            # persistent row buffers for the table build (pad stays zero)
            GB = 4   # build chunks per DMA group
            rowbufs = []
            for i in range(3):
                rb = ppool.tile([P, GB, 256], dt.bfloat16, tag=f"rowb{i}",
                                name=f"rowb{i}")
                nc.vector.memset(rb[:], 0.0)
                rowbufs.append(rb)

            def build_table(li):
                """Build gather table for layer li; returns own-region al_d."""
                L = LAYERS[li]
                H, C, FIN, ELEM = L["H"], L["C"], L["FIN"], L["ELEM"]
                HC = H * C
                ROW = HC + H
                t_tab = t_tab12 if li < 2 else t_tab3
                aldmy = ppool.tile([P, BCHUNK, H], dt.bfloat16, tag="aldmy",
                                   name="aldmy")
                nc.vector.memset(aldmy[:], 0.0)
                it = 0
                for r in range(N_CORES):
                    for c0 in range(0, BCHUNK, GB):
                        gn = min(GB, BCHUNK - c0)
                        n0 = r * RNODES + c0 * P
                        ntot = min(gn * P, RNODES - c0 * P)
                        lhsT = bpool.tile([FIN, GB * P], dt.bfloat16, tag="lhsT")
                        if li == 0:
                            nc.scalar.dma_start(lhsT[:, :ntot],
                                                t_xT[:, n0:n0 + ntot])
                        else:
                            lo, hi = c0 * P, c0 * P + ntot
                            if hi <= SPLITC:
                                nc.scalar.dma_start(
                                    lhsT[:, :ntot],
                                    t_agA[li - 1][r, :, lo:hi])
                            elif lo >= SPLITC:
                                nc.scalar.dma_start(
                                    lhsT[:, :ntot],
                                    t_agB[li - 1][r, :, lo - SPLITC:hi - SPLITC])
                            else:
                                nc.scalar.dma_start(
                                    lhsT[:, :SPLITC - lo],
                                    t_agA[li - 1][r, :, lo:SPLITC])
                                nc.scalar.dma_start(
                                    lhsT[:, SPLITC - lo:ntot],
                                    t_agB[li - 1][r, :, 0:hi - SPLITC])
                        rb = rowbufs[it % 3]
                        it += 1
                        for k in range(gn):
                            c = c0 + k
                            cols = min(P, RNODES - c * P)
                            hps = bps.tile([P, HC + 2 * H], dt.float32,
                                           space="PSUM", tag="hps", bufs=2)
                            nc.tensor.matmul(hps[:cols, :],
                                             lhsT[:, k * P:k * P + cols],
                                             CW_sb[li][:], start=True, stop=True)
                            # own-region al_d accumulate (sel mask) from PSUM
                            nc.vector.scalar_tensor_tensor(
                                out=aldmy[:cols, c, :],
                                in0=hps[:cols, HC + H:HC + 2 * H],
                                scalar=sel[:cols, r:r + 1],
                                in1=aldmy[:cols, c, :],
                                op0=OP.mult, op1=OP.add)
                            # table row [h | als] via scalar-engine PSUM drain
                            nc.scalar.activation(rb[:cols, k, 0:ROW],
                                                 hps[:cols, 0:ROW], AF.Copy)
                        if ntot % P == 0:
                            nc.sync.dma_start(
                                t_tab[n0:n0 + ntot, 0:ELEM].rearrange(
                                    "(n p) c -> p n c", p=P),
                                rb[:, 0:gn, 0:ELEM])
                        else:
                            for k in range(gn):
                                c = c0 + k
                                cols = min(P, RNODES - c * P)
                                nc.sync.dma_start(
                                    t_tab[r * RNODES + c * P:
                                          r * RNODES + c * P + cols, 0:ELEM],
                                    rb[:cols, k, 0:ELEM])
                return aldmy

            def aggregate(li, aldmy_sb, aldsw_sb):
                L = LAYERS[li]
                H, C, ELEM = L["H"], L["C"], L["ELEM"]
                HC = H * C
                ROW = HC + H
                RW = H + HC   # msg width: [w | w*h]
                t_tab = t_tab12 if li < 2 else t_tab3
                eluT = (ppool.tile([64, BCHUNK * P], dt.bfloat16, tag="eluT",
                                   name="eluT") if li < 2 else None)

                # --- per-window ald pair table, built once per layer ---
                # apw[:, w, :] = [ald_w(slots) on parts 0:64 in cols 0:H |
                #                 ald_w(slots) on parts 64:128 in cols H:2H]
                apw = ppool.tile([P, NWIN, 2 * H], dt.bfloat16, tag="apw",
                                 name="apw")
                apw_v = apw[:].rearrange("p (n t) h -> p t n h", t=2)
                nc.vector.memset(apw[0:W, :, H:2 * H], 0.0)
                nc.vector.memset(apw[W:P, :, 0:H], 0.0)
                nc.vector.tensor_copy(apw_v[0:W, 0, :, 0:H], aldmy_sb[0:W, :, :])
                nc.vector.tensor_copy(apw_v[0:W, 1, :, 0:H], aldsw_sb[0:W, :, :])
                nc.vector.tensor_copy(apw_v[W:P, 0, :, H:2 * H], aldsw_sb[W:P, :, :])
                nc.vector.tensor_copy(apw_v[W:P, 1, :, H:2 * H], aldmy_sb[W:P, :, :])

                for ci, cb in enumerate(btab):
                    ws, nAblk, nBblk = cb["ws"], cb["nAblk"], cb["nBblk"]
                    nblk = nAblk + nBblk
                    gb0 = cb["blocks"][0][3]

                    # ---- gathers: one tile, A blocks then B blocks ----
                    # (the SWDGE gather ucode rejects num_idxs > 1024: GMAX
                    # 12 and 16 both fault at runtime — keep 8 blocks max)
                    GMAX = 8
                    g = gpool.tile([P, CHUNK_BLOCKS, ELEM], dt.bfloat16, tag="g")
                    if nAblk:
                        for ob in range(0, nAblk, GMAX):
                            nb = min(GMAX, nAblk - ob)
                            o0 = cb["offA"] + ob * P
                            nc.gpsimd.dma_gather(
                                g[:, ob:ob + nb, :], t_tab[0:HALF, 0:ELEM],
                                idxA_sb[:, o0 // 16:(o0 + nb * P) // 16],
                                nb * P, nb * P, ELEM, queue_num=qc[0] % NQ)
                            qc[0] += 1
                    if nBblk:
                        for ob in range(0, nBblk, GMAX):
                            nb = min(GMAX, nBblk - ob)
                            o0 = cb["offB"] + ob * P
                            nc.gpsimd.dma_gather(
                                g[:, nAblk + ob:nAblk + ob + nb, :],
                                t_tab[HALF:N_NODES, 0:ELEM],
                                idxB_sb[:, o0 // 16:(o0 + nb * P) // 16],
                                nb * P, nb * P, ELEM, queue_num=qc[0] % NQ)
                            qc[0] += 1

                    # ---- batched one-hot build: one is_equal per chunk ----
                    oh_all = epool.tile([P, CHUNK_BLOCKS + 1, W],
                                        dt.bfloat16, tag="oh")
                    nc.vector.tensor_tensor(
                        out=oh_all[:, 0:nblk, :],
                        in0=dstcol_sb[:, gb0:gb0 + nblk].unsqueeze(-1)
                            .to_broadcast([P, nblk, W]),
                        in1=iota_row[:].unsqueeze(1).to_broadcast([P, nblk, W]),
                        op=OP.is_equal)
                    nc.vector.memset(oh_all[:, nblk, :], 0.0)

                    # ---- pairs within each (window, half) run ----
                    # lone pair at a run end: its second alde half spills into
                    # the next block's column and is overwritten by that
                    # block's own pair (in-order PE).
                    pairs = cb["pairs"]
                    npair = len(pairs)

                    # ---- pair transposes, 4 per PSUM tile, scalar drain ----
                    ohT_all = kpool.tile([P, max_npair, P],
                                         dt.bfloat16, tag="ohT", bufs=2)
                    for t0 in range(0, npair, 4):
                        nk = min(4, npair - t0)
                        tp = aps.tile([P, 512], dt.bfloat16, space="PSUM",
                                      tag="tp", bufs=2)
                        for k in range(nk):
                            lb0 = pairs[t0 + k][0]
                            nc.tensor.transpose(
                                tp[:, k * P:(k + 1) * P],
                                oh_all[:, lb0:lb0 + 2, :].rearrange(
                                    "p a b -> p (a b)"),
                                ident[:])
                        nc.scalar.activation(
                            ohT_all[:, t0:t0 + nk, :].rearrange("p n e -> p (n e)"),
                            tp[:, 0:nk * P], AF.Copy)

                    # ---- alde matmuls, block-aligned cols in one PSUM tile ----
                    aldeps = aps.tile([P, (CHUNK_BLOCKS + 2) * H],
                                      dt.float32, space="PSUM", tag="aldeps",
                                      bufs=2)
                    for pi, (lb0, w_) in enumerate(pairs):
                        nc.tensor.matmul(aldeps[:, lb0 * H:(lb0 + 2) * H],
                                         ohT_all[:, pi, :], apw[:, w_, :],
                                         start=True, stop=True)
                    alde_sb = epool.tile([P, (CHUNK_BLOCKS + 2) * H],
                                         dt.float32, tag="alde")
                    nc.scalar.activation(alde_sb[:, 0:nblk * H],
                                         aldeps[:, 0:nblk * H], AF.Copy)
                    alde_v = alde_sb[:].rearrange("p (n h) -> p n h", h=H)

                    # z = al_s + al_d ; lrelu ; w = exp  (batched per chunk)
                    zt = epool.tile([P, CHUNK_BLOCKS, H], dt.float32, tag="zt")
                    nc.vector.tensor_tensor(
                        out=zt[:, 0:nblk, :],
                        in0=g[:, 0:nblk, HC:HC + H],
                        in1=alde_v[:, 0:nblk, :], op=OP.add)
                    nc.vector.scalar_tensor_tensor(
                        out=zt[:, 0:nblk, :], in0=zt[:, 0:nblk, :], scalar=0.2,
                        in1=zt[:, 0:nblk, :], op0=OP.mult, op1=OP.max)
                    wa = epool.tile([P, CHUNK_BLOCKS, H], dt.bfloat16, tag="wa")
                    nc.scalar.activation(wa[:, 0:nblk, :], zt[:, 0:nblk, :],
                                         AF.Exp)

                    # ---- messages: m = [wa | h * wa] ----
                    m_t = epool.tile([P, CHUNK_BLOCKS, RW], dt.bfloat16, tag="m")
                    nc.scalar.activation(m_t[:, 0:nblk, 0:H], wa[:, 0:nblk, :],
                                         AF.Copy)
                    nc.vector.tensor_tensor(
                        out=m_t[:, 0:nblk, H:RW].rearrange(
                            "p n (h c) -> p n h c", c=C),
                        in0=g[:, 0:nblk, 0:HC].rearrange(
                            "p n (h c) -> p n h c", c=C),
                        in1=wa[:, 0:nblk, :].unsqueeze(-1).to_broadcast(
                            [P, nblk, H, C]),
                        op=OP.mult)

                    # ---- scatter per window ----
                    perwin = {}
                    for (w, half, lb, gbk) in cb["blocks"]:
                        perwin.setdefault(w, []).append(lb)
                    nw = len(ws)
                    eacc = wpool.tile([W, nw, RW], dt.float32, tag="eacc")
                    for wi, w in enumerate(ws):
                        blks = perwin[w]
                        acc = aps.tile([W, RW], dt.float32, space="PSUM",
                                       tag="acc", bufs=2)
                        for j, lb in enumerate(blks):
                            nc.tensor.matmul(acc[:], oh_all[:, lb, :],
                                             m_t[:, lb, :],
                                             start=(j == 0), stop=(j == len(blks) - 1))
                        wn = min(W, RNODES - w * W)
                        nc.scalar.activation(eacc[:wn, wi, :], acc[:wn, :],
                                             AF.Copy)

                    # ---- batched epilogue over the chunk's windows ----
                    rs = wpool.tile([W, nw, H], dt.float32, tag="rs")
                    nc.vector.reciprocal(rs[:], eacc[:, :, 0:H])
                    on = wpool.tile([W, nw, HC], dt.float32, tag="on")
                    nc.vector.tensor_tensor(
                        out=on[:].rearrange("p n (h c) -> p n h c", c=C),
                        in0=eacc[:, :, H:RW].rearrange("p n (h c) -> p n h c", c=C),
                        in1=rs[:].unsqueeze(-1).to_broadcast([W, nw, H, C]),
                        op=OP.mult)
                    if li < 2:
                        nc.vector.tensor_tensor(
                            out=on[:],
                            in0=on[:],
                            in1=b_sb[li][0:W, :].unsqueeze(1).to_broadcast([W, nw, HC]),
                            op=OP.add)
                        # elu = exp(min(x,0)) + max(x,0) - 1
                        # min(x,0) = -relu(-x); both steps on the scalar engine
                        zm = wpool.tile([W, nw, HC], dt.float32, tag="zm")
                        nc.scalar.activation(zm[:], on[:], AF.Relu, scale=-1.0)
                        ez = wpool.tile([W, nw, HC], dt.float32, tag="ez")
                        nc.scalar.activation(ez[:], zm[:], AF.Exp, scale=-1.0)
                        elf = wpool.tile([W, nw, HC], dt.float32, tag="elf")
                        nc.vector.scalar_tensor_tensor(
                            out=elf[:], in0=on[:], scalar=0.0,
                            in1=ez[:], op0=OP.max, op1=OP.add)
                        el = wpool.tile([W, nw, HC], dt.bfloat16, tag="el")
                        nc.scalar.activation(el[:], elf[:], AF.Copy, bias=-1.0)
                        # transpose each window -> eluT slice (scalar drain)
                        for wi, w in enumerate(ws):
                            wn = min(W, RNODES - w * W)
                            tps = aps.tile([P, 512], dt.bfloat16, space="PSUM",
                                           tag="tp", bufs=2)
                            nc.tensor.transpose(tps[:HC, :wn], el[:wn, wi, :],
                                                ident[:wn, :wn])
                            nc.scalar.activation(
                                eluT[:, w * W:w * W + wn], tps[:HC, :wn],
                                AF.Copy)
                        # first-half AllGather as soon as windows < SPLITW are
                        # done: lets build(li+1) start under this agg's tail
                        if ws[-1] >= SPLITW - 1 and (ci == 0 or
                                                     btab[ci - 1]["ws"][-1] < SPLITW - 1):
                            nc.sync.dma_start(t_ccinA[li][:, :],
                                              eluT[:, 0:SPLITC])
                            nc.gpsimd.collective_compute(
                                "AllGather", mybir.AluOpType.bypass,
                                replica_groups=[core_ids],
                                ins=[t_ccinA[li][:, :]],
                                outs=[t_agA[li][:, :, :].rearrange(
                                    "r p n -> (r p) n")],
                            )
                    else:
                        # mean over heads -> [*, nw, C]; 1/H folded into the
                        # head-sum via a scalar-engine scaled copy of on
                        ons = wpool.tile([W, nw, HC], dt.float32, tag="ons")
                        nc.scalar.activation(ons[:], on[:], AF.Copy,
                                             scale=1.0 / H)
                        mn = wpool.tile([W, nw, C], dt.float32, tag="mn")
                        nc.vector.tensor_reduce(
                            mn[:], ons[:].rearrange("p n (h c) -> p n c h", h=H),
                            axis=mybir.AxisListType.X, op=OP.add)
                        nc.vector.tensor_tensor(
                            out=mn[:], in0=mn[:],
                            in1=b_sb[2][0:W, :].unsqueeze(1).to_broadcast([W, nw, C]),
                            op=OP.add)
                        # elu sans the -1 (log_softmax is shift-invariant)
                        zm = wpool.tile([W, nw, C], dt.float32, tag="zm3")
                        nc.scalar.activation(zm[:], mn[:], AF.Relu, scale=-1.0)
                        ez = wpool.tile([W, nw, C], dt.float32, tag="ez3")
                        nc.scalar.activation(ez[:], zm[:], AF.Exp, scale=-1.0)
                        el = wpool.tile([W, nw, C], dt.float32, tag="el3")
                        nc.vector.scalar_tensor_tensor(
                            out=el[:], in0=mn[:], scalar=0.0,
                            in1=ez[:], op0=OP.max, op1=OP.add)
                        # log_softmax (batched)
                        mx = wpool.tile([W, nw, 1], dt.float32, tag="mx")
                        nc.vector.tensor_reduce(mx[:], el[:],
                                                axis=mybir.AxisListType.X, op=OP.max)
                        xm = wpool.tile([W, nw, C], dt.float32, tag="xm")
                        nc.vector.tensor_tensor(
                            out=xm[:], in0=el[:],
                            in1=mx[:].to_broadcast([W, nw, C]), op=OP.subtract)
                        ex = wpool.tile([W, nw, C], dt.float32, tag="ex3")
                        nc.scalar.activation(ex[:], xm[:], AF.Exp)
                        sm = wpool.tile([W, nw, 1], dt.float32, tag="sm")
                        nc.vector.tensor_reduce(sm[:], ex[:],
                                                axis=mybir.AxisListType.X, op=OP.add)
                        ls = wpool.tile([W, nw, 1], dt.float32, tag="ls")
                        nc.scalar.activation(ls[:], sm[:], AF.Ln)
                        fo = wpool.tile([W, nw, C], dt.float32, tag="fo")
                        nc.vector.tensor_tensor(
                            out=fo[:], in0=xm[:],
                            in1=ls[:].to_broadcast([W, nw, C]), op=OP.subtract)
                        # write out all full windows of the chunk
                        w0 = ws[0]
                        if ws[-1] * W + W <= RNODES:
                            nc.sync.dma_start(
                                t_out[w0 * W:ws[-1] * W + W, :].rearrange(
                                    "(n p) c -> p n c", p=W),
                                fo[:, :, :])
                        else:
                            for wi, w in enumerate(ws):
                                wn = min(W, RNODES - w * W)
                                nc.sync.dma_start(t_out[w * W:w * W + wn, :],
                                                  fo[:wn, wi, :])
                if li < 2:
                    # second-half AllGather (cols SPLITC:) — the first half
                    # was emitted mid-loop so the next layer's table build
                    # can overlap this layer's aggregation tail.
                    nc.sync.dma_start(t_ccinB[li][:, :],
                                      eluT[:, SPLITC:])
                    nc.gpsimd.collective_compute(
                        "AllGather", mybir.AluOpType.bypass,
                        replica_groups=[core_ids],
                        ins=[t_ccinB[li][:, :]],
                        outs=[t_agB[li][:, :, :].rearrange(
                            "r p n -> (r p) n")],
                    )

            for li in range(3):
                aldmy_sb = build_table(li)
                H_li = LAYERS[li]["H"]
                aldsw_sb = ppool.tile([P, BCHUNK, H_li], dt.bfloat16,
                                      tag="aldsw", name="aldsw")
                nc.sync.dma_start(aldsw_sb[0:64, :, :], aldmy_sb[64:128, :, :])
                nc.sync.dma_start(aldsw_sb[64:128, :, :], aldmy_sb[0:64, :, :])
                aggregate(li, aldmy_sb, aldsw_sb)

    nc.compile()
    return nc


def prepare(inputs):
    meta, percore = _preprocess(np.asarray(inputs["edge_index"]))
    wd = _prep_weights(inputs)
    nc = build_program(meta)

    in_maps = []
    for d in range(N_CORES):
        sel = np.zeros((P, N_CORES), np.float32)
        sel[:, d] = 1.0
        m = dict(
            xT=wd["xT"],
            sel=sel,
            idxA=percore[d]["idxA"], idxB=percore[d]["idxB"],
            dstcol=percore[d]["dstcol"],
        )
        for i in range(3):
            m[f"CW{i}"] = wd[f"CW{i}"]
            m[f"b{i}"] = wd[f"b{i}"]
        in_maps.append(m)
    return nc, in_maps


def kernel(x, edge_index, W1, a1s, a1d, b1, W2, a2s, a2d, b2, W3, a3s, a3d, b3):
    from concourse.bass_utils import run_bass_kernel_spmd

    inputs = dict(x=x, edge_index=edge_index, W1=W1, a1s=a1s, a1d=a1d, b1=b1,
                  W2=W2, a2s=a2s, a2d=a2d, b2=b2, W3=W3, a3s=a3s, a3d=a3d, b3=b3)
    nc, in_maps = prepare(inputs)
    res = run_bass_kernel_spmd(nc, in_maps, core_ids=list(range(N_CORES)))
    out = np.concatenate([res.results[d]["out"] for d in range(N_CORES)], axis=0)
    return out.astype(np.float32)


# revision 28
# speedup vs baseline: 1.1807x; 1.1807x over previous
"""3-layer GAT (50k nodes, 1.6M edges) on 8 Trainium2 NeuronCores — v3.

v2 edge-parallel-by-destination strategy, with the engine-level hotspots
rebalanced based on the NTFF profile (Vector 85% busy, GpSimd ~100% busy
during aggregation, Scalar 3%):
  - gathers rotate across 4 SWDGE queues (was 1).
  - one-hot build: ONE batched is_equal per chunk for all blocks.
  - pair transposes: 4 per PSUM tile, drained by the Scalar engine.
  - alde matmuls col-sliced into ONE PSUM tile per chunk, one scalar drain.
  - single gather tile per chunk (A+B) -> single zt/lrelu/exp/msg ops.
  - table rows are [h | als] (no ones columns); the per-edge weight lands
    in the message tile via a Scalar-engine copy instead.
  - PSUM->SBUF drains (eacc, eluT, alde, ohT, table rows) moved to the
    Scalar engine; DVE keeps only the arithmetic.
"""

import numpy as np
import ml_dtypes

P = 128
N_NODES = 50000
N_EDGES = 1600000
F_IN = 128
N_CORES = 8
RNODES = N_NODES // N_CORES          # 6250 nodes per core
W = 64                               # scatter window (nodes)
NWIN = (RNODES + W - 1) // W         # 98 windows per core
BCHUNK = (RNODES + P - 1) // P       # 49 table-build chunks per region
HALF = 25000                         # gather table half size (int16 limit)
CHUNK_BLOCKS = 48                    # max 128-edge blocks per aggregation chunk
NQ = 4                               # SWDGE queues for gathers

# per-layer (H, C, ELEM): ELEM = bf16 elements per table row (256B aligned)
# row layout: [h (H*C) | als (H) | pad]
LAYERS = [
    dict(H=4, C=16, FIN=128, ELEM=128),
    dict(H=4, C=16, FIN=64, ELEM=128),
    dict(H=6, C=40, FIN=64, ELEM=256),
]

BF16 = ml_dtypes.bfloat16


def _preprocess(edge_index):
    """Sort edges by dst, split per core / window / table-half, pad each
    (window, half) group to a uniform (across cores) multiple of 128."""
    src = np.concatenate([edge_index[0], np.arange(N_NODES, dtype=np.int64)])
    dst = np.concatenate([edge_index[1], np.arange(N_NODES, dtype=np.int64)])
    order = np.argsort(dst, kind="stable")
    src = src[order].astype(np.int64)
    dst = dst[order].astype(np.int64)

    nwh = N_CORES * NWIN
    counts = np.zeros((N_CORES, NWIN, 2), np.int64)
    groups = [[None, None] for _ in range(nwh)]
    for d in range(N_CORES):
        lo_d = d * RNODES
        for w in range(NWIN):
            a = lo_d + w * W
            b = min(lo_d + (w + 1) * W, lo_d + RNODES)
            i0 = np.searchsorted(dst, a)
            i1 = np.searchsorted(dst, b)
            s = src[i0:i1]
            t = dst[i0:i1]
            selA = s < HALF
            groups[d * NWIN + w][0] = (s[selA], t[selA] - a)
            groups[d * NWIN + w][1] = (s[~selA] - HALF, t[~selA] - a)
            counts[d, w, 0] = int(selA.sum())
            counts[d, w, 1] = int((~selA).sum())

    mA = np.ceil(counts[:, :, 0].max(axis=0) / P).astype(int)   # [NWIN]
    mB = np.ceil(counts[:, :, 1].max(axis=0) / P).astype(int)

    chunks = []
    cur, cur_blocks = [], 0
    for w in range(NWIN):
        blk = int(mA[w] + mB[w])
        assert blk <= CHUNK_BLOCKS, f"window {w} too big: {blk} blocks"
        if cur_blocks + blk > CHUNK_BLOCKS:
            chunks.append(cur)
            cur, cur_blocks = [], 0
        cur.append(w)
        cur_blocks += blk
    if cur:
        chunks.append(cur)

    totA = int(mA.sum()) * P
    totB = int(mB.sum()) * P
    nblk_tot = int(mA.sum() + mB.sum())

    meta = dict(mA=mA, mB=mB, chunks=chunks, totA=totA, totB=totB,
                nblk_tot=nblk_tot)

    percore = []
    for d in range(N_CORES):
        eA = np.zeros(totA, np.int16)
        eB = np.zeros(totB, np.int16)
        dstcol = np.full(nblk_tot * P, 99.0, BF16)
        pa = pb = 0
        gb = 0
        for ws in chunks:
            for w in ws:
                s, tl = groups[d * NWIN + w][0]
                n = len(s)
                eA[pa:pa + n] = s
                dstcol[gb * P: gb * P + n] = tl.astype(BF16)
                pa += mA[w] * P
                gb += int(mA[w])
            for w in ws:
                s, tl = groups[d * NWIN + w][1]
                n = len(s)
                eB[pb:pb + n] = s
                dstcol[gb * P: gb * P + n] = tl.astype(BF16)
                pb += mB[w] * P
                gb += int(mB[w])
        idxA = np.tile(eA.reshape(-1, 16).T, (8, 1)) if totA else np.zeros((128, 0), np.int16)
        idxB = np.tile(eB.reshape(-1, 16).T, (8, 1)) if totB else np.zeros((128, 0), np.int16)
        percore.append(dict(
            idxA=np.ascontiguousarray(idxA),
            idxB=np.ascontiguousarray(idxB),
            dstcol=np.ascontiguousarray(dstcol.reshape(nblk_tot, P).T),
        ))
    return meta, percore


def _block_table(meta):
    """Per chunk: block layout [A-blocks by window | B-blocks by window]."""
    mA, mB, chunks = meta["mA"], meta["mB"], meta["chunks"]
    out = []
    gb = 0
    offA = offB = 0
    for ws in chunks:
        nAblk = int(sum(mA[w] for w in ws))
        nBblk = int(sum(mB[w] for w in ws))
        blocks = []   # (w, half, local_block_in_chunk, global_block)
        lb = 0
        for w in ws:
            for _ in range(int(mA[w])):
                blocks.append((w, 0, lb, gb)); lb += 1; gb += 1
        for w in ws:
            for _ in range(int(mB[w])):
                blocks.append((w, 1, lb, gb)); lb += 1; gb += 1
        # pairs within each (window, half) run; lone pair at odd run ends
        pairs = []   # (local_block_of_first, window)
        i = 0
        nblk = len(blocks)
        while i < nblk:
            w_i, half_i = blocks[i][0], blocks[i][1]
            j = i
            while j < nblk and blocks[j][0] == w_i and blocks[j][1] == half_i:
                j += 1
            for k in range(i, j, 2):
                pairs.append((k, w_i))
            i = j
        out.append(dict(ws=ws, nAblk=nAblk, nBblk=nBblk, blocks=blocks,
                        offA=offA, offB=offB, pairs=pairs))
        offA += nAblk * P
        offB += nBblk * P
    return out


def _prep_weights(inputs):
    """Host-side constant prep: combined [W | W@As | W@Ad] per layer."""
    x = np.asarray(inputs["x"], np.float32)
    d = {}
    d["xT"] = np.ascontiguousarray(x.T).astype(BF16)            # [128, 50000]
    for i, (wk, ask, adk, bk) in enumerate(
            [("W1", "a1s", "a1d", "b1"), ("W2", "a2s", "a2d", "b2"),
             ("W3", "a3s", "a3d", "b3")]):
        L = LAYERS[i]
        H, C = L["H"], L["C"]
        Wm = np.asarray(inputs[wk], np.float32)                  # [FIN, H*C]
        a_s = np.asarray(inputs[ask], np.float32)                # [H, C]
        a_d = np.asarray(inputs[adk], np.float32)
        b = np.asarray(inputs[bk], np.float32)
        # als[n,h] = sum_c h[n,h,c]*a_s[h,c] = h @ Asm, Asm[h*C+c, h]=a_s[h,c]
        Asm = np.zeros((H * C, H), np.float32)
        Adm = np.zeros((H * C, H), np.float32)
        for h in range(H):
            Asm[h * C:(h + 1) * C, h] = a_s[h]
            Adm[h * C:(h + 1) * C, h] = a_d[h]
        CW = np.concatenate([Wm, Wm @ Asm, Wm @ Adm], axis=1)    # [FIN, HC+2H]
        d[f"CW{i}"] = CW.astype(BF16)
        d[f"b{i}"] = np.tile(b.reshape(1, -1), (P, 1)).astype(np.float32)
    return d


def build_program(meta, n_cores=None):
    import concourse.bacc as bacc
    import concourse.bass as bass
    import concourse.tile as tile
    import concourse.mybir as mybir
    from concourse.masks import make_identity

    dt = mybir.dt
    AF = mybir.ActivationFunctionType
    OP = mybir.AluOpType

    if n_cores is None:
        n_cores = N_CORES
    mA, mB, chunks = meta["mA"], meta["mB"], meta["chunks"]
    totA, totB, nblk_tot = meta["totA"], meta["totB"], meta["nblk_tot"]
    btab = _block_table(meta)
    max_npair = max(len(cb["pairs"]) for cb in btab)

    nc = bacc.Bacc("TRN2", target_bir_lowering=False, debug=False,
                   num_swdge_queues=NQ)

    # ---------------- I/O ----------------
    t_xT = nc.dram_tensor("xT", [P, N_NODES], dt.bfloat16, kind="ExternalInput")
    t_CW = [nc.dram_tensor(f"CW{i}", [LAYERS[i]["FIN"],
                                      LAYERS[i]["H"] * LAYERS[i]["C"] + 2 * LAYERS[i]["H"]],
                           dt.bfloat16, kind="ExternalInput") for i in range(3)]
    t_b = [nc.dram_tensor(f"b{i}", [P, LAYERS[i]["H"] * LAYERS[i]["C"] if i < 2 else LAYERS[i]["C"]],
                          dt.float32, kind="ExternalInput") for i in range(3)]
    t_sel = nc.dram_tensor("sel", [P, N_CORES], dt.float32, kind="ExternalInput")
    t_idxA = nc.dram_tensor("idxA", [P, max(totA // 16, 1)], dt.int16, kind="ExternalInput")
    t_idxB = nc.dram_tensor("idxB", [P, max(totB // 16, 1)], dt.int16, kind="ExternalInput")
    t_dstcol = nc.dram_tensor("dstcol", [P, nblk_tot], dt.bfloat16, kind="ExternalInput")
    t_out = nc.dram_tensor("out", [RNODES, LAYERS[2]["C"]], dt.float32, kind="ExternalOutput")

    # ---------------- internal DRAM ----------------
    t_tab12 = nc.dram_tensor("tab12", [N_NODES, 128], dt.bfloat16)
    t_tab3 = nc.dram_tensor("tab3", [N_NODES, 256], dt.bfloat16)
    # (a split mid-aggregation AllGather was tried and regressed: the
    # collective on the gpsimd queue stalls the gather stream while all
    # cores sync — keep the single end-of-layer collective)
    t_ccin = [nc.dram_tensor(f"ccin{i}", [64, BCHUNK * P], dt.bfloat16) for i in range(2)]
    t_ag = [nc.dram_tensor(f"ag{i}", [N_CORES, 64, BCHUNK * P], dt.bfloat16,
                           addr_space="Shared") for i in range(2)]

    core_ids = list(range(n_cores))
    qc = [0]   # gather queue rotation counter

    with tile.TileContext(nc) as tc:
        with tc.tile_pool(name="const", bufs=1) as cpool, \
             tc.tile_pool(name="persist", bufs=1) as ppool, \
             tc.tile_pool(name="build", bufs=3) as bpool, \
             tc.tile_pool(name="gath", bufs=2) as gpool, \
             tc.tile_pool(name="edge", bufs=2) as epool, \
             tc.tile_pool(name="blk", bufs=6) as kpool, \
             tc.tile_pool(name="win", bufs=2) as wpool, \
             tc.tile_pool(name="bps", bufs=2, space="PSUM") as bps, \
             tc.tile_pool(name="aps", bufs=2, space="PSUM") as aps:

            # ----- constants -----
            ident = cpool.tile([P, P], dt.bfloat16, tag="ident")
            make_identity(nc, ident[:])
            iota_i = cpool.tile([P, W], dt.int32, tag="iotai")
            nc.gpsimd.iota(iota_i[:], [[1, W]], base=0, channel_multiplier=0)
            iota_row = cpool.tile([P, W], dt.bfloat16, tag="iotarow")
            nc.vector.tensor_copy(iota_row[:], iota_i[:])
            sel = cpool.tile([P, N_CORES], dt.float32, tag="sel")
            nc.sync.dma_start(sel[:], t_sel[:])
            CW_sb, b_sb = [], []
            for i in range(3):
                L = LAYERS[i]
                HC2H = L["H"] * L["C"] + 2 * L["H"]
                wt = cpool.tile([L["FIN"], HC2H], dt.bfloat16, tag=f"CW{i}")
                nc.sync.dma_start(wt[:], t_CW[i][:])
                CW_sb.append(wt)
                bt = cpool.tile([P, L["H"] * L["C"] if i < 2 else L["C"]], dt.float32, tag=f"b{i}")
                nc.sync.dma_start(bt[:], t_b[i][:])
                b_sb.append(bt)

            # ----- resident edge data -----
            idxA_sb = ppool.tile([P, max(totA // 16, 1)], dt.int16, tag="idxA")
            nc.sync.dma_start(idxA_sb[:], t_idxA[:])
            idxB_sb = ppool.tile([P, max(totB // 16, 1)], dt.int16, tag="idxB")
            nc.sync.dma_start(idxB_sb[:], t_idxB[:])
            dstcol_sb = ppool.tile([P, nblk_tot], dt.bfloat16, tag="dstcol")
            nc.sync.dma_start(dstcol_sb[:], t_dstcol[:])

            # persistent row buffers for the table build (pad stays zero)
            GB = 4   # build chunks per DMA group
            rowbufs = []
            for i in range(3):
                rb = ppool.tile([P, GB, 256], dt.bfloat16, tag=f"rowb{i}",
                                name=f"rowb{i}")
                nc.vector.memset(rb[:], 0.0)
                rowbufs.append(rb)

            def build_table(li):
                """Build gather table for layer li; returns own-region al_d."""
                L = LAYERS[li]
                H, C, FIN, ELEM = L["H"], L["C"], L["FIN"], L["ELEM"]
                HC = H * C
                ROW = HC + H
                t_tab = t_tab12 if li < 2 else t_tab3
                aldmy = ppool.tile([P, BCHUNK, H], dt.bfloat16, tag="aldmy",
                                   name="aldmy")
                nc.vector.memset(aldmy[:], 0.0)
                it = 0
                for r in range(N_CORES):
                    for c0 in range(0, BCHUNK, GB):
                        gn = min(GB, BCHUNK - c0)
                        n0 = r * RNODES + c0 * P
                        ntot = min(gn * P, RNODES - c0 * P)
                        lhsT = bpool.tile([FIN, GB * P], dt.bfloat16, tag="lhsT")
                        if li == 0:
                            nc.scalar.dma_start(lhsT[:, :ntot],
                                                t_xT[:, n0:n0 + ntot])
                        else:
                            nc.scalar.dma_start(
                                lhsT[:, :ntot],
                                t_ag[li - 1][r, :, c0 * P:c0 * P + ntot])
                        rb = rowbufs[it % 3]
                        it += 1
                        for k in range(gn):
                            c = c0 + k
                            cols = min(P, RNODES - c * P)
                            hps = bps.tile([P, HC + 2 * H], dt.float32,
                                           space="PSUM", tag="hps", bufs=2)
                            nc.tensor.matmul(hps[:cols, :],
                                             lhsT[:, k * P:k * P + cols],
                                             CW_sb[li][:], start=True, stop=True)
                            # own-region al_d accumulate (sel mask) from PSUM
                            nc.vector.scalar_tensor_tensor(
                                out=aldmy[:cols, c, :],
                                in0=hps[:cols, HC + H:HC + 2 * H],
                                scalar=sel[:cols, r:r + 1],
                                in1=aldmy[:cols, c, :],
                                op0=OP.mult, op1=OP.add)
                            # table row [h | als] via scalar-engine PSUM drain
                            nc.scalar.activation(rb[:cols, k, 0:ROW],
                                                 hps[:cols, 0:ROW], AF.Copy)
                        if ntot % P == 0:
                            nc.sync.dma_start(
                                t_tab[n0:n0 + ntot, 0:ELEM].rearrange(
                                    "(n p) c -> p n c", p=P),
                                rb[:, 0:gn, 0:ELEM])
                        else:
                            for k in range(gn):
                                c = c0 + k
                                cols = min(P, RNODES - c * P)
                                nc.sync.dma_start(
                                    t_tab[r * RNODES + c * P:
                                          r * RNODES + c * P + cols, 0:ELEM],
                                    rb[:cols, k, 0:ELEM])
                return aldmy

            def aggregate(li, aldmy_sb, aldsw_sb):
                L = LAYERS[li]
                H, C, ELEM = L["H"], L["C"], L["ELEM"]
                HC = H * C
                ROW = HC + H
                RW = H + HC   # msg width: [w | w*h]
                t_tab = t_tab12 if li < 2 else t_tab3
                eluT = (ppool.tile([64, BCHUNK * P], dt.bfloat16, tag="eluT",
                                   name="eluT") if li < 2 else None)

                # --- per-window ald pair table, built once per layer ---
                # apw[:, w, :] = [ald_w(slots) on parts 0:64 in cols 0:H |
                #                 ald_w(slots) on parts 64:128 in cols H:2H]
                apw = ppool.tile([P, NWIN, 2 * H], dt.bfloat16, tag="apw",
                                 name="apw")
                apw_v = apw[:].rearrange("p (n t) h -> p t n h", t=2)
                nc.vector.memset(apw[0:W, :, H:2 * H], 0.0)
                nc.vector.memset(apw[W:P, :, 0:H], 0.0)
                nc.vector.tensor_copy(apw_v[0:W, 0, :, 0:H], aldmy_sb[0:W, :, :])
                nc.vector.tensor_copy(apw_v[0:W, 1, :, 0:H], aldsw_sb[0:W, :, :])
                nc.vector.tensor_copy(apw_v[W:P, 0, :, H:2 * H], aldsw_sb[W:P, :, :])
                nc.vector.tensor_copy(apw_v[W:P, 1, :, H:2 * H], aldmy_sb[W:P, :, :])

                for ci, cb in enumerate(btab):
                    ws, nAblk, nBblk = cb["ws"], cb["nAblk"], cb["nBblk"]
                    nblk = nAblk + nBblk
                    gb0 = cb["blocks"][0][3]

                    # ---- gathers: one tile, A blocks then B blocks ----
                    # (the SWDGE gather ucode rejects num_idxs > 1024: GMAX
                    # 12 and 16 both fault at runtime — keep 8 blocks max)
                    GMAX = 8
                    g = gpool.tile([P, CHUNK_BLOCKS, ELEM], dt.bfloat16, tag="g")
                    if nAblk:
                        for ob in range(0, nAblk, GMAX):
                            nb = min(GMAX, nAblk - ob)
                            o0 = cb["offA"] + ob * P
                            nc.gpsimd.dma_gather(
                                g[:, ob:ob + nb, :], t_tab[0:HALF, 0:ELEM],
                                idxA_sb[:, o0 // 16:(o0 + nb * P) // 16],
                                nb * P, nb * P, ELEM, queue_num=qc[0] % NQ)
                            qc[0] += 1
                    if nBblk:
                        for ob in range(0, nBblk, GMAX):
                            nb = min(GMAX, nBblk - ob)
                            o0 = cb["offB"] + ob * P
                            nc.gpsimd.dma_gather(
                                g[:, nAblk + ob:nAblk + ob + nb, :],
                                t_tab[HALF:N_NODES, 0:ELEM],
                                idxB_sb[:, o0 // 16:(o0 + nb * P) // 16],
                                nb * P, nb * P, ELEM, queue_num=qc[0] % NQ)
                            qc[0] += 1

                    # ---- batched one-hot build: one is_equal per chunk ----
                    oh_all = epool.tile([P, CHUNK_BLOCKS + 1, W],
                                        dt.bfloat16, tag="oh")
                    nc.vector.tensor_tensor(
                        out=oh_all[:, 0:nblk, :],
                        in0=dstcol_sb[:, gb0:gb0 + nblk].unsqueeze(-1)
                            .to_broadcast([P, nblk, W]),
                        in1=iota_row[:].unsqueeze(1).to_broadcast([P, nblk, W]),
                        op=OP.is_equal)
                    nc.vector.memset(oh_all[:, nblk, :], 0.0)

                    # ---- pairs within each (window, half) run ----
                    # lone pair at a run end: its second alde half spills into
                    # the next block's column and is overwritten by that
                    # block's own pair (in-order PE).
                    pairs = cb["pairs"]
                    npair = len(pairs)

                    # ---- pair transposes, 4 per PSUM tile, scalar drain ----
                    ohT_all = kpool.tile([P, max_npair, P],
                                         dt.bfloat16, tag="ohT", bufs=2)
                    for t0 in range(0, npair, 4):
                        nk = min(4, npair - t0)
                        tp = aps.tile([P, 512], dt.bfloat16, space="PSUM",
                                      tag="tp", bufs=2)
                        for k in range(nk):
                            lb0 = pairs[t0 + k][0]
                            nc.tensor.transpose(
                                tp[:, k * P:(k + 1) * P],
                                oh_all[:, lb0:lb0 + 2, :].rearrange(
                                    "p a b -> p (a b)"),
                                ident[:])
                        nc.scalar.activation(
                            ohT_all[:, t0:t0 + nk, :].rearrange("p n e -> p (n e)"),
                            tp[:, 0:nk * P], AF.Copy)

                    # ---- alde matmuls, block-aligned cols in one PSUM tile ----
                    aldeps = aps.tile([P, (CHUNK_BLOCKS + 2) * H],
                                      dt.float32, space="PSUM", tag="aldeps",
                                      bufs=2)
                    for pi, (lb0, w_) in enumerate(pairs):
                        nc.tensor.matmul(aldeps[:, lb0 * H:(lb0 + 2) * H],
                                         ohT_all[:, pi, :], apw[:, w_, :],
                                         start=True, stop=True)
                    alde_sb = epool.tile([P, (CHUNK_BLOCKS + 2) * H],
                                         dt.float32, tag="alde")
                    nc.scalar.activation(alde_sb[:, 0:nblk * H],
                                         aldeps[:, 0:nblk * H], AF.Copy)
                    alde_v = alde_sb[:].rearrange("p (n h) -> p n h", h=H)

                    # z = al_s + al_d ; lrelu ; w = exp  (batched per chunk)
                    zt = epool.tile([P, CHUNK_BLOCKS, H], dt.float32, tag="zt")
                    nc.vector.tensor_tensor(
                        out=zt[:, 0:nblk, :],
                        in0=g[:, 0:nblk, HC:HC + H],
                        in1=alde_v[:, 0:nblk, :], op=OP.add)
                    nc.vector.scalar_tensor_tensor(
                        out=zt[:, 0:nblk, :], in0=zt[:, 0:nblk, :], scalar=0.2,
                        in1=zt[:, 0:nblk, :], op0=OP.mult, op1=OP.max)
                    wa = epool.tile([P, CHUNK_BLOCKS, H], dt.bfloat16, tag="wa")
                    nc.scalar.activation(wa[:, 0:nblk, :], zt[:, 0:nblk, :],
                                         AF.Exp)

                    # ---- messages: m = [wa | h * wa] ----
                    m_t = epool.tile([P, CHUNK_BLOCKS, RW], dt.bfloat16, tag="m")
                    nc.scalar.activation(m_t[:, 0:nblk, 0:H], wa[:, 0:nblk, :],
                                         AF.Copy)
                    nc.vector.tensor_tensor(
                        out=m_t[:, 0:nblk, H:RW].rearrange(
                            "p n (h c) -> p n h c", c=C),
                        in0=g[:, 0:nblk, 0:HC].rearrange(
                            "p n (h c) -> p n h c", c=C),
                        in1=wa[:, 0:nblk, :].unsqueeze(-1).to_broadcast(
                            [P, nblk, H, C]),
                        op=OP.mult)

                    # ---- scatter per window ----
                    perwin = {}
                    for (w, half, lb, gbk) in cb["blocks"]:
                        perwin.setdefault(w, []).append(lb)
                    nw = len(ws)
                    eacc = wpool.tile([W, nw, RW], dt.float32, tag="eacc")
                    for wi, w in enumerate(ws):
                        blks = perwin[w]
                        acc = aps.tile([W, RW], dt.float32, space="PSUM",
                                       tag="acc", bufs=2)
                        for j, lb in enumerate(blks):
                            nc.tensor.matmul(acc[:], oh_all[:, lb, :],
                                             m_t[:, lb, :],
                                             start=(j == 0), stop=(j == len(blks) - 1))
                        wn = min(W, RNODES - w * W)
                        nc.scalar.activation(eacc[:wn, wi, :], acc[:wn, :],
                                             AF.Copy)

                    # ---- batched epilogue over the chunk's windows ----
                    rs = wpool.tile([W, nw, H], dt.float32, tag="rs")
                    nc.vector.reciprocal(rs[:], eacc[:, :, 0:H])
                    on = wpool.tile([W, nw, HC], dt.float32, tag="on")
                    nc.vector.tensor_tensor(
                        out=on[:].rearrange("p n (h c) -> p n h c", c=C),
                        in0=eacc[:, :, H:RW].rearrange("p n (h c) -> p n h c", c=C),
                        in1=rs[:].unsqueeze(-1).to_broadcast([W, nw, H, C]),
                        op=OP.mult)
                    if li < 2:
                        nc.vector.tensor_tensor(
                            out=on[:],
                            in0=on[:],
                            in1=b_sb[li][0:W, :].unsqueeze(1).to_broadcast([W, nw, HC]),
                            op=OP.add)
                        # elu = exp(min(x,0)) + max(x,0) - 1
                        # min(x,0) = -relu(-x); both steps on the scalar engine
                        zm = wpool.tile([W, nw, HC], dt.float32, tag="zm")
                        nc.scalar.activation(zm[:], on[:], AF.Relu, scale=-1.0)
                        ez = wpool.tile([W, nw, HC], dt.float32, tag="ez")
                        nc.scalar.activation(ez[:], zm[:], AF.Exp, scale=-1.0)
                        elf = wpool.tile([W, nw, HC], dt.float32, tag="elf")
                        nc.vector.scalar_tensor_tensor(
                            out=elf[:], in0=on[:], scalar=0.0,
                            in1=ez[:], op0=OP.max, op1=OP.add)
                        el = wpool.tile([W, nw, HC], dt.bfloat16, tag="el")
                        nc.scalar.activation(el[:], elf[:], AF.Copy, bias=-1.0)
                        # transpose each window -> eluT slice (scalar drain)
                        for wi, w in enumerate(ws):
                            wn = min(W, RNODES - w * W)
                            tps = aps.tile([P, 512], dt.bfloat16, space="PSUM",
                                           tag="tp", bufs=2)
                            nc.tensor.transpose(tps[:HC, :wn], el[:wn, wi, :],
                                                ident[:wn, :wn])
                            nc.scalar.activation(
                                eluT[:, w * W:w * W + wn], tps[:HC, :wn],
                                AF.Copy)
                    else:
                        # mean over heads -> [*, nw, C]; 1/H folded into the
                        # head-sum via a scalar-engine scaled copy of on
                        ons = wpool.tile([W, nw, HC], dt.float32, tag="ons")
                        nc.scalar.activation(ons[:], on[:], AF.Copy,
                                             scale=1.0 / H)
                        mn = wpool.tile([W, nw, C], dt.float32, tag="mn")
                        nc.vector.tensor_reduce(
                            mn[:], ons[:].rearrange("p n (h c) -> p n c h", h=H),
                            axis=mybir.AxisListType.X, op=OP.add)
                        nc.vector.tensor_tensor(
                            out=mn[:], in0=mn[:],
                            in1=b_sb[2][0:W, :].unsqueeze(1).to_broadcast([W, nw, C]),
                            op=OP.add)
                        # elu sans the -1 (log_softmax is shift-invariant)
                        zm = wpool.tile([W, nw, C], dt.float32, tag="zm3")
                        nc.scalar.activation(zm[:], mn[:], AF.Relu, scale=-1.0)
                        ez = wpool.tile([W, nw, C], dt.float32, tag="ez3")
                        nc.scalar.activation(ez[:], zm[:], AF.Exp, scale=-1.0)
                        el = wpool.tile([W, nw, C], dt.float32, tag="el3")
                        nc.vector.scalar_tensor_tensor(
                            out=el[:], in0=mn[:], scalar=0.0,
                            in1=ez[:], op0=OP.max, op1=OP.add)
                        # log_softmax (batched)
                        mx = wpool.tile([W, nw, 1], dt.float32, tag="mx")
                        nc.vector.tensor_reduce(mx[:], el[:],
                                                axis=mybir.AxisListType.X, op=OP.max)
                        xm = wpool.tile([W, nw, C], dt.float32, tag="xm")
                        nc.vector.tensor_tensor(
                            out=xm[:], in0=el[:],
                            in1=mx[:].to_broadcast([W, nw, C]), op=OP.subtract)
                        ex = wpool.tile([W, nw, C], dt.float32, tag="ex3")
                        nc.scalar.activation(ex[:], xm[:], AF.Exp)
                        sm = wpool.tile([W, nw, 1], dt.float32, tag="sm")
                        nc.vector.tensor_reduce(sm[:], ex[:],
                                                axis=mybir.AxisListType.X, op=OP.add)
                        ls = wpool.tile([W, nw, 1], dt.float32, tag="ls")
                        nc.scalar.activation(ls[:], sm[:], AF.Ln)
                        fo = wpool.tile([W, nw, C], dt.float32, tag="fo")
                        nc.vector.tensor_tensor(
                            out=fo[:], in0=xm[:],
                            in1=ls[:].to_broadcast([W, nw, C]), op=OP.subtract)
                        # write out all full windows of the chunk
                        w0 = ws[0]
                        if ws[-1] * W + W <= RNODES:
                            nc.sync.dma_start(
                                t_out[w0 * W:ws[-1] * W + W, :].rearrange(
                                    "(n p) c -> p n c", p=W),
                                fo[:, :, :])
                        else:
                            for wi, w in enumerate(ws):
                                wn = min(W, RNODES - w * W)
                                nc.sync.dma_start(t_out[w * W:w * W + wn, :],
                                                  fo[:wn, wi, :])
                if li < 2:
                    nc.sync.dma_start(t_ccin[li][:, :], eluT[:])
                    nc.gpsimd.collective_compute(
                        "AllGather", mybir.AluOpType.bypass,
                        replica_groups=[core_ids],
                        ins=[t_ccin[li][:, :]],
                        outs=[t_ag[li][:, :, :].rearrange("r p n -> (r p) n")],
                    )

            for li in range(3):
                aldmy_sb = build_table(li)
                H_li = LAYERS[li]["H"]
                aldsw_sb = ppool.tile([P, BCHUNK, H_li], dt.bfloat16,
                                      tag="aldsw", name="aldsw")
                nc.sync.dma_start(aldsw_sb[0:64, :, :], aldmy_sb[64:128, :, :])
                nc.sync.dma_start(aldsw_sb[64:128, :, :], aldmy_sb[0:64, :, :])
                aggregate(li, aldmy_sb, aldsw_sb)

    nc.compile()
    return nc


def prepare(inputs):
    meta, percore = _preprocess(np.asarray(inputs["edge_index"]))
    wd = _prep_weights(inputs)
    nc = build_program(meta)

    in_maps = []
    for d in range(N_CORES):
        sel = np.zeros((P, N_CORES), np.float32)
        sel[:, d] = 1.0
        m = dict(
            xT=wd["xT"],
            sel=sel,
            idxA=percore[d]["idxA"], idxB=percore[d]["idxB"],
            dstcol=percore[d]["dstcol"],
        )
        for i in range(3):
            m[f"CW{i}"] = wd[f"CW{i}"]
            m[f"b{i}"] = wd[f"b{i}"]
        in_maps.append(m)
    return nc, in_maps


def kernel(x, edge_index, W1, a1s, a1d, b1, W2, a2s, a2d, b2, W3, a3s, a3d, b3):
    from concourse.bass_utils import run_bass_kernel_spmd

    inputs = dict(x=x, edge_index=edge_index, W1=W1, a1s=a1s, a1d=a1d, b1=b1,
                  W2=W2, a2s=a2s, a2d=a2d, b2=b2, W3=W3, a3s=a3s, a3d=a3d, b3=b3)
    nc, in_maps = prepare(inputs)
    res = run_bass_kernel_spmd(nc, in_maps, core_ids=list(range(N_CORES)))
    out = np.concatenate([res.results[d]["out"] for d in range(N_CORES)], axis=0)
    return out.astype(np.float32)


# revision 33
# speedup vs baseline: 1.1876x; 1.0058x over previous
"""3-layer GAT (50k nodes, 1.6M edges) on 8 Trainium2 NeuronCores — v3.

v2 edge-parallel-by-destination strategy, with the engine-level hotspots
rebalanced based on the NTFF profile (Vector 85% busy, GpSimd ~100% busy
during aggregation, Scalar 3%):
  - gathers rotate across 4 SWDGE queues (was 1).
  - one-hot build: ONE batched is_equal per chunk for all blocks.
  - pair transposes: 4 per PSUM tile, drained by the Scalar engine.
  - alde matmuls col-sliced into ONE PSUM tile per chunk, one scalar drain.
  - single gather tile per chunk (A+B) -> single zt/lrelu/exp/msg ops.
  - table rows are [h | als] (no ones columns); the per-edge weight lands
    in the message tile via a Scalar-engine copy instead.
  - PSUM->SBUF drains (eacc, eluT, alde, ohT, table rows) moved to the
    Scalar engine; DVE keeps only the arithmetic.
"""

import numpy as np
import ml_dtypes

P = 128
N_NODES = 50000
N_EDGES = 1600000
F_IN = 128
N_CORES = 8
RNODES = N_NODES // N_CORES          # 6250 nodes per core
W = 64                               # scatter window (nodes)
NWIN = (RNODES + W - 1) // W         # 98 windows per core
BCHUNK = (RNODES + P - 1) // P       # 49 table-build chunks per region
HALF = 25000                         # gather table half size (int16 limit)
CHUNK_BLOCKS = 48                    # max 128-edge blocks per aggregation chunk
NQ = 4                               # SWDGE queues for gathers
SPLITW = 80                          # eluT AllGather split point (windows)
SPLITC = SPLITW * W                  # ... in eluT columns (= 40 chunks)

# per-layer (H, C, ELEM): ELEM = bf16 elements per table row (256B aligned)
# row layout: [h (H*C) | als (H) | pad]
LAYERS = [
    dict(H=4, C=16, FIN=128, ELEM=128),
    dict(H=4, C=16, FIN=64, ELEM=128),
    dict(H=6, C=40, FIN=64, ELEM=256),
]

BF16 = ml_dtypes.bfloat16


def _preprocess(edge_index):
    """Sort edges by dst, split per core / window / table-half, pad each
    (window, half) group to a uniform (across cores) multiple of 128."""
    src = np.concatenate([edge_index[0], np.arange(N_NODES, dtype=np.int64)])
    dst = np.concatenate([edge_index[1], np.arange(N_NODES, dtype=np.int64)])
    order = np.argsort(dst, kind="stable")
    src = src[order].astype(np.int64)
    dst = dst[order].astype(np.int64)

    nwh = N_CORES * NWIN
    counts = np.zeros((N_CORES, NWIN, 2), np.int64)
    groups = [[None, None] for _ in range(nwh)]
    for d in range(N_CORES):
        lo_d = d * RNODES
        for w in range(NWIN):
            a = lo_d + w * W
            b = min(lo_d + (w + 1) * W, lo_d + RNODES)
            i0 = np.searchsorted(dst, a)
            i1 = np.searchsorted(dst, b)
            s = src[i0:i1]
            t = dst[i0:i1]
            selA = s < HALF
            groups[d * NWIN + w][0] = (s[selA], t[selA] - a)
            groups[d * NWIN + w][1] = (s[~selA] - HALF, t[~selA] - a)
            counts[d, w, 0] = int(selA.sum())
            counts[d, w, 1] = int((~selA).sum())

    mA = np.ceil(counts[:, :, 0].max(axis=0) / P).astype(int)   # [NWIN]
    mB = np.ceil(counts[:, :, 1].max(axis=0) / P).astype(int)

    chunks = []
    cur, cur_blocks = [], 0
    for w in range(NWIN):
        blk = int(mA[w] + mB[w])
        assert blk <= CHUNK_BLOCKS, f"window {w} too big: {blk} blocks"
        if cur_blocks + blk > CHUNK_BLOCKS:
            chunks.append(cur)
            cur, cur_blocks = [], 0
        cur.append(w)
        cur_blocks += blk
    if cur:
        chunks.append(cur)

    totA = int(mA.sum()) * P
    totB = int(mB.sum()) * P
    nblk_tot = int(mA.sum() + mB.sum())

    meta = dict(mA=mA, mB=mB, chunks=chunks, totA=totA, totB=totB,
                nblk_tot=nblk_tot)

    percore = []
    for d in range(N_CORES):
        eA = np.zeros(totA, np.int16)
        eB = np.zeros(totB, np.int16)
        dstcol = np.full(nblk_tot * P, 99.0, BF16)
        pa = pb = 0
        gb = 0
        for ws in chunks:
            for w in ws:
                s, tl = groups[d * NWIN + w][0]
                n = len(s)
                eA[pa:pa + n] = s
                dstcol[gb * P: gb * P + n] = tl.astype(BF16)
                pa += mA[w] * P
                gb += int(mA[w])
            for w in ws:
                s, tl = groups[d * NWIN + w][1]
                n = len(s)
                eB[pb:pb + n] = s
                dstcol[gb * P: gb * P + n] = tl.astype(BF16)
                pb += mB[w] * P
                gb += int(mB[w])
        idxA = np.tile(eA.reshape(-1, 16).T, (8, 1)) if totA else np.zeros((128, 0), np.int16)
        idxB = np.tile(eB.reshape(-1, 16).T, (8, 1)) if totB else np.zeros((128, 0), np.int16)
        percore.append(dict(
            idxA=np.ascontiguousarray(idxA),
            idxB=np.ascontiguousarray(idxB),
            dstcol=np.ascontiguousarray(dstcol.reshape(nblk_tot, P).T),
        ))
    return meta, percore


def _block_table(meta):
    """Per chunk: block layout [A-blocks by window | B-blocks by window]."""
    mA, mB, chunks = meta["mA"], meta["mB"], meta["chunks"]
    out = []
    gb = 0
    offA = offB = 0
    for ws in chunks:
        nAblk = int(sum(mA[w] for w in ws))
        nBblk = int(sum(mB[w] for w in ws))
        blocks = []   # (w, half, local_block_in_chunk, global_block)
        lb = 0
        for w in ws:
            for _ in range(int(mA[w])):
                blocks.append((w, 0, lb, gb)); lb += 1; gb += 1
        for w in ws:
            for _ in range(int(mB[w])):
                blocks.append((w, 1, lb, gb)); lb += 1; gb += 1
        # pairs within each (window, half) run; lone pair at odd run ends
        pairs = []   # (local_block_of_first, window)
        i = 0
        nblk = len(blocks)
        while i < nblk:
            w_i, half_i = blocks[i][0], blocks[i][1]
            j = i
            while j < nblk and blocks[j][0] == w_i and blocks[j][1] == half_i:
                j += 1
            for k in range(i, j, 2):
                pairs.append((k, w_i))
            i = j
        out.append(dict(ws=ws, nAblk=nAblk, nBblk=nBblk, blocks=blocks,
                        offA=offA, offB=offB, pairs=pairs))
        offA += nAblk * P
        offB += nBblk * P
    return out


def _prep_weights(inputs):
    """Host-side constant prep: combined [W | W@As | W@Ad] per layer."""
    x = np.asarray(inputs["x"], np.float32)
    d = {}
    d["xT"] = np.ascontiguousarray(x.T).astype(BF16)            # [128, 50000]
    for i, (wk, ask, adk, bk) in enumerate(
            [("W1", "a1s", "a1d", "b1"), ("W2", "a2s", "a2d", "b2"),
             ("W3", "a3s", "a3d", "b3")]):
        L = LAYERS[i]
        H, C = L["H"], L["C"]
        Wm = np.asarray(inputs[wk], np.float32)                  # [FIN, H*C]
        a_s = np.asarray(inputs[ask], np.float32)                # [H, C]
        a_d = np.asarray(inputs[adk], np.float32)
        b = np.asarray(inputs[bk], np.float32)
        # als[n,h] = sum_c h[n,h,c]*a_s[h,c] = h @ Asm, Asm[h*C+c, h]=a_s[h,c]
        Asm = np.zeros((H * C, H), np.float32)
        Adm = np.zeros((H * C, H), np.float32)
        for h in range(H):
            Asm[h * C:(h + 1) * C, h] = a_s[h]
            Adm[h * C:(h + 1) * C, h] = a_d[h]
        CW = np.concatenate([Wm, Wm @ Asm, Wm @ Adm], axis=1)    # [FIN, HC+2H]
        d[f"CW{i}"] = CW.astype(BF16)
        d[f"b{i}"] = np.tile(b.reshape(1, -1), (P, 1)).astype(np.float32)
    return d


def build_program(meta, n_cores=None):
    import concourse.bacc as bacc
    import concourse.bass as bass
    import concourse.tile as tile
    import concourse.mybir as mybir
    from concourse.masks import make_identity

    dt = mybir.dt
    AF = mybir.ActivationFunctionType
    OP = mybir.AluOpType

    if n_cores is None:
        n_cores = N_CORES
    mA, mB, chunks = meta["mA"], meta["mB"], meta["chunks"]
    totA, totB, nblk_tot = meta["totA"], meta["totB"], meta["nblk_tot"]
    btab = _block_table(meta)
    max_npair = max(len(cb["pairs"]) for cb in btab)

    nc = bacc.Bacc("TRN2", target_bir_lowering=False, debug=False,
                   num_swdge_queues=NQ)

    # ---------------- I/O ----------------
    t_xT = nc.dram_tensor("xT", [P, N_NODES], dt.bfloat16, kind="ExternalInput")
    t_CW = [nc.dram_tensor(f"CW{i}", [LAYERS[i]["FIN"],
                                      LAYERS[i]["H"] * LAYERS[i]["C"] + 2 * LAYERS[i]["H"]],
                           dt.bfloat16, kind="ExternalInput") for i in range(3)]
    t_b = [nc.dram_tensor(f"b{i}", [P, LAYERS[i]["H"] * LAYERS[i]["C"] if i < 2 else LAYERS[i]["C"]],
                          dt.float32, kind="ExternalInput") for i in range(3)]
    t_sel = nc.dram_tensor("sel", [P, N_CORES], dt.float32, kind="ExternalInput")
    t_idxA = nc.dram_tensor("idxA", [P, max(totA // 16, 1)], dt.int16, kind="ExternalInput")
    t_idxB = nc.dram_tensor("idxB", [P, max(totB // 16, 1)], dt.int16, kind="ExternalInput")
    t_dstcol = nc.dram_tensor("dstcol", [P, nblk_tot], dt.bfloat16, kind="ExternalInput")
    t_out = nc.dram_tensor("out", [RNODES, LAYERS[2]["C"]], dt.float32, kind="ExternalOutput")

    # ---------------- internal DRAM ----------------
    t_tab12 = nc.dram_tensor("tab12", [N_NODES, 128], dt.bfloat16)
    t_tab3 = nc.dram_tensor("tab3", [N_NODES, 256], dt.bfloat16)
    # eluT AllGather split into two contiguous buffer pairs. BOTH
    # collectives are emitted after the layer's gathers (a mid-loop
    # collective stalls the gather stream while cores sync — measured
    # +0.6ms), but the first one's input DMA fires mid-loop, so it
    # executes right after the last gather retires, before the trailing
    # epilogues finish — letting most of the next table build start early.
    NCOLS2 = BCHUNK * P - SPLITC
    t_ccinA = [nc.dram_tensor(f"ccinA{i}", [64, SPLITC], dt.bfloat16) for i in range(2)]
    t_ccinB = [nc.dram_tensor(f"ccinB{i}", [64, NCOLS2], dt.bfloat16) for i in range(2)]
    t_agA = [nc.dram_tensor(f"agA{i}", [N_CORES, 64, SPLITC], dt.bfloat16,
                            addr_space="Shared") for i in range(2)]
    t_agB = [nc.dram_tensor(f"agB{i}", [N_CORES, 64, NCOLS2], dt.bfloat16,
                            addr_space="Shared") for i in range(2)]

    core_ids = list(range(n_cores))
    qc = [0]   # gather queue rotation counter

    with tile.TileContext(nc) as tc:
        with tc.tile_pool(name="const", bufs=1) as cpool, \
             tc.tile_pool(name="persist", bufs=1) as ppool, \
             tc.tile_pool(name="build", bufs=3) as bpool, \
             tc.tile_pool(name="gath", bufs=2) as gpool, \
             tc.tile_pool(name="edge", bufs=2) as epool, \
             tc.tile_pool(name="blk", bufs=6) as kpool, \
             tc.tile_pool(name="win", bufs=2) as wpool, \
             tc.tile_pool(name="bps", bufs=2, space="PSUM") as bps, \
             tc.tile_pool(name="aps", bufs=2, space="PSUM") as aps:

            # ----- constants -----
            ident = cpool.tile([P, P], dt.bfloat16, tag="ident")
            make_identity(nc, ident[:])
            iota_i = cpool.tile([P, W], dt.int32, tag="iotai")
            nc.gpsimd.iota(iota_i[:], [[1, W]], base=0, channel_multiplier=0)
            iota_row = cpool.tile([P, W], dt.bfloat16, tag="iotarow")
            nc.vector.tensor_copy(iota_row[:], iota_i[:])
            sel = cpool.tile([P, N_CORES], dt.float32, tag="sel")
            nc.sync.dma_start(sel[:], t_sel[:])
            CW_sb, b_sb = [], []
            for i in range(3):
                L = LAYERS[i]
                HC2H = L["H"] * L["C"] + 2 * L["H"]
                wt = cpool.tile([L["FIN"], HC2H], dt.bfloat16, tag=f"CW{i}")
                nc.sync.dma_start(wt[:], t_CW[i][:])
                CW_sb.append(wt)
                bt = cpool.tile([P, L["H"] * L["C"] if i < 2 else L["C"]], dt.float32, tag=f"b{i}")
                nc.sync.dma_start(bt[:], t_b[i][:])
                b_sb.append(bt)

            # ----- resident edge data -----
            idxA_sb = ppool.tile([P, max(totA // 16, 1)], dt.int16, tag="idxA")
            nc.sync.dma_start(idxA_sb[:], t_idxA[:])
            idxB_sb = ppool.tile([P, max(totB // 16, 1)], dt.int16, tag="idxB")
            nc.sync.dma_start(idxB_sb[:], t_idxB[:])
            dstcol_sb = ppool.tile([P, nblk_tot], dt.bfloat16, tag="dstcol")
            nc.sync.dma_start(dstcol_sb[:], t_dstcol[:])

            # persistent row buffers for the table build (pad stays zero)
            GB = 4   # build chunks per DMA group
            rowbufs = []
            for i in range(3):
                rb = ppool.tile([P, GB, 256], dt.bfloat16, tag=f"rowb{i}",
                                name=f"rowb{i}")
                nc.vector.memset(rb[:], 0.0)
                rowbufs.append(rb)

            def build_table(li):
                """Build gather table for layer li; returns own-region al_d."""
                L = LAYERS[li]
                H, C, FIN, ELEM = L["H"], L["C"], L["FIN"], L["ELEM"]
                HC = H * C
                ROW = HC + H
                t_tab = t_tab12 if li < 2 else t_tab3
                aldmy = ppool.tile([P, BCHUNK, H], dt.bfloat16, tag="aldmy",
                                   name="aldmy")
                nc.vector.memset(aldmy[:], 0.0)
                it = 0
                for r in range(N_CORES):
                    for c0 in range(0, BCHUNK, GB):
                        gn = min(GB, BCHUNK - c0)
                        n0 = r * RNODES + c0 * P
                        ntot = min(gn * P, RNODES - c0 * P)
                        lhsT = bpool.tile([FIN, GB * P], dt.bfloat16, tag="lhsT")
                        if li == 0:
                            nc.scalar.dma_start(lhsT[:, :ntot],
                                                t_xT[:, n0:n0 + ntot])
                        else:
                            lo, hi = c0 * P, c0 * P + ntot
                            if hi <= SPLITC:
                                nc.scalar.dma_start(
                                    lhsT[:, :ntot],
                                    t_agA[li - 1][r, :, lo:hi])
                            elif lo >= SPLITC:
                                nc.scalar.dma_start(
                                    lhsT[:, :ntot],
                                    t_agB[li - 1][r, :, lo - SPLITC:hi - SPLITC])
                            else:
                                nc.scalar.dma_start(
                                    lhsT[:, :SPLITC - lo],
                                    t_agA[li - 1][r, :, lo:SPLITC])
                                nc.scalar.dma_start(
                                    lhsT[:, SPLITC - lo:ntot],
                                    t_agB[li - 1][r, :, 0:hi - SPLITC])
                        rb = rowbufs[it % 3]
                        it += 1
                        for k in range(gn):
                            c = c0 + k
                            cols = min(P, RNODES - c * P)
                            hps = bps.tile([P, HC + 2 * H], dt.float32,
                                           space="PSUM", tag="hps", bufs=2)
                            nc.tensor.matmul(hps[:cols, :],
                                             lhsT[:, k * P:k * P + cols],
                                             CW_sb[li][:], start=True, stop=True)
                            # own-region al_d accumulate (sel mask) from PSUM
                            nc.vector.scalar_tensor_tensor(
                                out=aldmy[:cols, c, :],
                                in0=hps[:cols, HC + H:HC + 2 * H],
                                scalar=sel[:cols, r:r + 1],
                                in1=aldmy[:cols, c, :],
                                op0=OP.mult, op1=OP.add)
                            # table row [h | als] via scalar-engine PSUM drain
                            nc.scalar.activation(rb[:cols, k, 0:ROW],
                                                 hps[:cols, 0:ROW], AF.Copy)
                        if ntot % P == 0:
                            nc.sync.dma_start(
                                t_tab[n0:n0 + ntot, 0:ELEM].rearrange(
                                    "(n p) c -> p n c", p=P),
                                rb[:, 0:gn, 0:ELEM])
                        else:
                            for k in range(gn):
                                c = c0 + k
                                cols = min(P, RNODES - c * P)
                                nc.sync.dma_start(
                                    t_tab[r * RNODES + c * P:
                                          r * RNODES + c * P + cols, 0:ELEM],
                                    rb[:cols, k, 0:ELEM])
                return aldmy

            def aggregate(li, aldmy_sb, aldsw_sb):
                L = LAYERS[li]
                H, C, ELEM = L["H"], L["C"], L["ELEM"]
                HC = H * C
                ROW = HC + H
                RW = H + HC   # msg width: [w | w*h]
                t_tab = t_tab12 if li < 2 else t_tab3
                eluT = (ppool.tile([64, BCHUNK * P], dt.bfloat16, tag="eluT",
                                   name="eluT") if li < 2 else None)

                # --- per-window ald pair table, built once per layer ---
                # apw[:, w, :] = [ald_w(slots) on parts 0:64 in cols 0:H |
                #                 ald_w(slots) on parts 64:128 in cols H:2H]
                apw = ppool.tile([P, NWIN, 2 * H], dt.bfloat16, tag="apw",
                                 name="apw")
                apw_v = apw[:].rearrange("p (n t) h -> p t n h", t=2)
                nc.vector.memset(apw[0:W, :, H:2 * H], 0.0)
                nc.vector.memset(apw[W:P, :, 0:H], 0.0)
                nc.vector.tensor_copy(apw_v[0:W, 0, :, 0:H], aldmy_sb[0:W, :, :])
                nc.vector.tensor_copy(apw_v[0:W, 1, :, 0:H], aldsw_sb[0:W, :, :])
                nc.vector.tensor_copy(apw_v[W:P, 0, :, H:2 * H], aldsw_sb[W:P, :, :])
                nc.vector.tensor_copy(apw_v[W:P, 1, :, H:2 * H], aldmy_sb[W:P, :, :])

                for ci, cb in enumerate(btab):
                    ws, nAblk, nBblk = cb["ws"], cb["nAblk"], cb["nBblk"]
                    nblk = nAblk + nBblk
                    gb0 = cb["blocks"][0][3]

                    # ---- gathers: one tile, A blocks then B blocks ----
                    # (the SWDGE gather ucode rejects num_idxs > 1024: GMAX
                    # 12 and 16 both fault at runtime — keep 8 blocks max)
                    GMAX = 8
                    g = gpool.tile([P, CHUNK_BLOCKS, ELEM], dt.bfloat16, tag="g")
                    if nAblk:
                        for ob in range(0, nAblk, GMAX):
                            nb = min(GMAX, nAblk - ob)
                            o0 = cb["offA"] + ob * P
                            nc.gpsimd.dma_gather(
                                g[:, ob:ob + nb, :], t_tab[0:HALF, 0:ELEM],
                                idxA_sb[:, o0 // 16:(o0 + nb * P) // 16],
                                nb * P, nb * P, ELEM, queue_num=qc[0] % NQ)
                            qc[0] += 1
                    if nBblk:
                        for ob in range(0, nBblk, GMAX):
                            nb = min(GMAX, nBblk - ob)
                            o0 = cb["offB"] + ob * P
                            nc.gpsimd.dma_gather(
                                g[:, nAblk + ob:nAblk + ob + nb, :],
                                t_tab[HALF:N_NODES, 0:ELEM],
                                idxB_sb[:, o0 // 16:(o0 + nb * P) // 16],
                                nb * P, nb * P, ELEM, queue_num=qc[0] % NQ)
                            qc[0] += 1

                    # ---- batched one-hot build: one is_equal per chunk ----
                    oh_all = epool.tile([P, CHUNK_BLOCKS + 1, W],
                                        dt.bfloat16, tag="oh")
                    nc.vector.tensor_tensor(
                        out=oh_all[:, 0:nblk, :],
                        in0=dstcol_sb[:, gb0:gb0 + nblk].unsqueeze(-1)
                            .to_broadcast([P, nblk, W]),
                        in1=iota_row[:].unsqueeze(1).to_broadcast([P, nblk, W]),
                        op=OP.is_equal)
                    nc.vector.memset(oh_all[:, nblk, :], 0.0)

                    # ---- pairs within each (window, half) run ----
                    # lone pair at a run end: its second alde half spills into
                    # the next block's column and is overwritten by that
                    # block's own pair (in-order PE).
                    pairs = cb["pairs"]
                    npair = len(pairs)

                    # ---- pair transposes, 4 per PSUM tile, scalar drain ----
                    ohT_all = kpool.tile([P, max_npair, P],
                                         dt.bfloat16, tag="ohT", bufs=2)
                    for t0 in range(0, npair, 4):
                        nk = min(4, npair - t0)
                        tp = aps.tile([P, 512], dt.bfloat16, space="PSUM",
                                      tag="tp", bufs=2)
                        for k in range(nk):
                            lb0 = pairs[t0 + k][0]
                            nc.tensor.transpose(
                                tp[:, k * P:(k + 1) * P],
                                oh_all[:, lb0:lb0 + 2, :].rearrange(
                                    "p a b -> p (a b)"),
                                ident[:])
                        nc.scalar.activation(
                            ohT_all[:, t0:t0 + nk, :].rearrange("p n e -> p (n e)"),
                            tp[:, 0:nk * P], AF.Copy)

                    # ---- alde matmuls, block-aligned cols in one PSUM tile ----
                    aldeps = aps.tile([P, (CHUNK_BLOCKS + 2) * H],
                                      dt.float32, space="PSUM", tag="aldeps",
                                      bufs=2)
                    for pi, (lb0, w_) in enumerate(pairs):
                        nc.tensor.matmul(aldeps[:, lb0 * H:(lb0 + 2) * H],
                                         ohT_all[:, pi, :], apw[:, w_, :],
                                         start=True, stop=True)
                    alde_sb = epool.tile([P, (CHUNK_BLOCKS + 2) * H],
                                         dt.float32, tag="alde")
                    nc.scalar.activation(alde_sb[:, 0:nblk * H],
                                         aldeps[:, 0:nblk * H], AF.Copy)
                    alde_v = alde_sb[:].rearrange("p (n h) -> p n h", h=H)

                    # z = al_s + al_d ; lrelu ; w = exp  (batched per chunk)
                    zt = epool.tile([P, CHUNK_BLOCKS, H], dt.float32, tag="zt")
                    nc.vector.tensor_tensor(
                        out=zt[:, 0:nblk, :],
                        in0=g[:, 0:nblk, HC:HC + H],
                        in1=alde_v[:, 0:nblk, :], op=OP.add)
                    nc.vector.scalar_tensor_tensor(
                        out=zt[:, 0:nblk, :], in0=zt[:, 0:nblk, :], scalar=0.2,
                        in1=zt[:, 0:nblk, :], op0=OP.mult, op1=OP.max)
                    wa = epool.tile([P, CHUNK_BLOCKS, H], dt.bfloat16, tag="wa")
                    nc.scalar.activation(wa[:, 0:nblk, :], zt[:, 0:nblk, :],
                                         AF.Exp)

                    # ---- messages: m = [wa | h * wa] ----
                    m_t = epool.tile([P, CHUNK_BLOCKS, RW], dt.bfloat16, tag="m")
                    nc.scalar.activation(m_t[:, 0:nblk, 0:H], wa[:, 0:nblk, :],
                                         AF.Copy)
                    nc.vector.tensor_tensor(
                        out=m_t[:, 0:nblk, H:RW].rearrange(
                            "p n (h c) -> p n h c", c=C),
                        in0=g[:, 0:nblk, 0:HC].rearrange(
                            "p n (h c) -> p n h c", c=C),
                        in1=wa[:, 0:nblk, :].unsqueeze(-1).to_broadcast(
                            [P, nblk, H, C]),
                        op=OP.mult)

                    # ---- scatter per window ----
                    perwin = {}
                    for (w, half, lb, gbk) in cb["blocks"]:
                        perwin.setdefault(w, []).append(lb)
                    nw = len(ws)
                    eacc = wpool.tile([W, nw, RW], dt.float32, tag="eacc")
                    for wi, w in enumerate(ws):
                        blks = perwin[w]
                        acc = aps.tile([W, RW], dt.float32, space="PSUM",
                                       tag="acc", bufs=2)
                        for j, lb in enumerate(blks):
                            nc.tensor.matmul(acc[:], oh_all[:, lb, :],
                                             m_t[:, lb, :],
                                             start=(j == 0), stop=(j == len(blks) - 1))
                        wn = min(W, RNODES - w * W)
                        nc.scalar.activation(eacc[:wn, wi, :], acc[:wn, :],
                                             AF.Copy)

                    # ---- batched epilogue over the chunk's windows ----
                    rs = wpool.tile([W, nw, H], dt.float32, tag="rs")
                    nc.vector.reciprocal(rs[:], eacc[:, :, 0:H])
                    on = wpool.tile([W, nw, HC], dt.float32, tag="on")
                    nc.vector.tensor_tensor(
                        out=on[:].rearrange("p n (h c) -> p n h c", c=C),
                        in0=eacc[:, :, H:RW].rearrange("p n (h c) -> p n h c", c=C),
                        in1=rs[:].unsqueeze(-1).to_broadcast([W, nw, H, C]),
                        op=OP.mult)
                    if li < 2:
                        nc.vector.tensor_tensor(
                            out=on[:],
                            in0=on[:],
                            in1=b_sb[li][0:W, :].unsqueeze(1).to_broadcast([W, nw, HC]),
                            op=OP.add)
                        # elu = exp(min(x,0)) + max(x,0) - 1
                        # min(x,0) = -relu(-x); both steps on the scalar engine
                        zm = wpool.tile([W, nw, HC], dt.float32, tag="zm")
                        nc.scalar.activation(zm[:], on[:], AF.Relu, scale=-1.0)
                        ez = wpool.tile([W, nw, HC], dt.float32, tag="ez")
                        nc.scalar.activation(ez[:], zm[:], AF.Exp, scale=-1.0)
                        elf = wpool.tile([W, nw, HC], dt.float32, tag="elf")
                        nc.vector.scalar_tensor_tensor(
                            out=elf[:], in0=on[:], scalar=0.0,
                            in1=ez[:], op0=OP.max, op1=OP.add)
                        el = wpool.tile([W, nw, HC], dt.bfloat16, tag="el")
                        nc.scalar.activation(el[:], elf[:], AF.Copy, bias=-1.0)
                        # transpose each window -> eluT slice (scalar drain)
                        for wi, w in enumerate(ws):
                            wn = min(W, RNODES - w * W)
                            tps = aps.tile([P, 512], dt.bfloat16, space="PSUM",
                                           tag="tp", bufs=2)
                            nc.tensor.transpose(tps[:HC, :wn], el[:wn, wi, :],
                                                ident[:wn, :wn])
                            nc.scalar.activation(
                                eluT[:, w * W:w * W + wn], tps[:HC, :wn],
                                AF.Copy)
                        # stage the first AllGather's input as soon as
                        # windows < SPLITW are transposed (sync queue only)
                        if ws[-1] >= SPLITW - 1 and (ci == 0 or
                                                     btab[ci - 1]["ws"][-1] < SPLITW - 1):
                            nc.sync.dma_start(t_ccinA[li][:, :],
                                              eluT[:, 0:SPLITC])
                    else:
                        # mean over heads -> [*, nw, C]; 1/H folded into the
                        # head-sum via a scalar-engine scaled copy of on
                        ons = wpool.tile([W, nw, HC], dt.float32, tag="ons")
                        nc.scalar.activation(ons[:], on[:], AF.Copy,
                                             scale=1.0 / H)
                        mn = wpool.tile([W, nw, C], dt.float32, tag="mn")
                        nc.vector.tensor_reduce(
                            mn[:], ons[:].rearrange("p n (h c) -> p n c h", h=H),
                            axis=mybir.AxisListType.X, op=OP.add)
                        nc.vector.tensor_tensor(
                            out=mn[:], in0=mn[:],
                            in1=b_sb[2][0:W, :].unsqueeze(1).to_broadcast([W, nw, C]),
                            op=OP.add)
                        # elu sans the -1 (log_softmax is shift-invariant)
                        zm = wpool.tile([W, nw, C], dt.float32, tag="zm3")
                        nc.scalar.activation(zm[:], mn[:], AF.Relu, scale=-1.0)
                        ez = wpool.tile([W, nw, C], dt.float32, tag="ez3")
                        nc.scalar.activation(ez[:], zm[:], AF.Exp, scale=-1.0)
                        el = wpool.tile([W, nw, C], dt.float32, tag="el3")
                        nc.vector.scalar_tensor_tensor(
                            out=el[:], in0=mn[:], scalar=0.0,
                            in1=ez[:], op0=OP.max, op1=OP.add)
                        # log_softmax (batched)
                        mx = wpool.tile([W, nw, 1], dt.float32, tag="mx")
                        nc.vector.tensor_reduce(mx[:], el[:],
                                                axis=mybir.AxisListType.X, op=OP.max)
                        xm = wpool.tile([W, nw, C], dt.float32, tag="xm")
                        nc.vector.tensor_tensor(
                            out=xm[:], in0=el[:],
                            in1=mx[:].to_broadcast([W, nw, C]), op=OP.subtract)
                        ex = wpool.tile([W, nw, C], dt.float32, tag="ex3")
                        nc.scalar.activation(ex[:], xm[:], AF.Exp)
                        sm = wpool.tile([W, nw, 1], dt.float32, tag="sm")
                        nc.vector.tensor_reduce(sm[:], ex[:],
                                                axis=mybir.AxisListType.X, op=OP.add)
                        ls = wpool.tile([W, nw, 1], dt.float32, tag="ls")
                        nc.scalar.activation(ls[:], sm[:], AF.Ln)
                        fo = wpool.tile([W, nw, C], dt.float32, tag="fo")
                        nc.vector.tensor_tensor(
                            out=fo[:], in0=xm[:],
                            in1=ls[:].to_broadcast([W, nw, C]), op=OP.subtract)
                        # write out all full windows of the chunk
                        w0 = ws[0]
                        if ws[-1] * W + W <= RNODES:
                            nc.sync.dma_start(
                                t_out[w0 * W:ws[-1] * W + W, :].rearrange(
                                    "(n p) c -> p n c", p=W),
                                fo[:, :, :])
                        else:
                            for wi, w in enumerate(ws):
                                wn = min(W, RNODES - w * W)
                                nc.sync.dma_start(t_out[w * W:w * W + wn, :],
                                                  fo[:wn, wi, :])
                if li < 2:
                    nc.gpsimd.collective_compute(
                        "AllGather", mybir.AluOpType.bypass,
                        replica_groups=[core_ids],
                        ins=[t_ccinA[li][:, :]],
                        outs=[t_agA[li][:, :, :].rearrange("r p n -> (r p) n")],
                    )
                    nc.sync.dma_start(t_ccinB[li][:, :], eluT[:, SPLITC:])
                    nc.gpsimd.collective_compute(
                        "AllGather", mybir.AluOpType.bypass,
                        replica_groups=[core_ids],
                        ins=[t_ccinB[li][:, :]],
                        outs=[t_agB[li][:, :, :].rearrange("r p n -> (r p) n")],
                    )

            for li in range(3):
                aldmy_sb = build_table(li)
                H_li = LAYERS[li]["H"]
                aldsw_sb = ppool.tile([P, BCHUNK, H_li], dt.bfloat16,
                                      tag="aldsw", name="aldsw")
                nc.sync.dma_start(aldsw_sb[0:64, :, :], aldmy_sb[64:128, :, :])
                nc.sync.dma_start(aldsw_sb[64:128, :, :], aldmy_sb[0:64, :, :])
                aggregate(li, aldmy_sb, aldsw_sb)

    nc.compile()
    return nc


def prepare(inputs):
    meta, percore = _preprocess(np.asarray(inputs["edge_index"]))
    wd = _prep_weights(inputs)
    nc = build_program(meta)

    in_maps = []
    for d in range(N_CORES):
        sel = np.zeros((P, N_CORES), np.float32)
        sel[:, d] = 1.0
        m = dict(
            xT=wd["xT"],
            sel=sel,
            idxA=percore[d]["idxA"], idxB=percore[d]["idxB"],
            dstcol=percore[d]["dstcol"],
        )
        for i in range(3):
            m[f"CW{i}"] = wd[f"CW{i}"]
            m[f"b{i}"] = wd[f"b{i}"]
        in_maps.append(m)
    return nc, in_maps


def kernel(x, edge_index, W1, a1s, a1d, b1, W2, a2s, a2d, b2, W3, a3s, a3d, b3):
    from concourse.bass_utils import run_bass_kernel_spmd

    inputs = dict(x=x, edge_index=edge_index, W1=W1, a1s=a1s, a1d=a1d, b1=b1,
                  W2=W2, a2s=a2s, a2d=a2d, b2=b2, W3=W3, a3s=a3s, a3d=a3d, b3=b3)
    nc, in_maps = prepare(inputs)
    res = run_bass_kernel_spmd(nc, in_maps, core_ids=list(range(N_CORES)))
    out = np.concatenate([res.results[d]["out"] for d in range(N_CORES)], axis=0)
    return out.astype(np.float32)


# revision 35
# speedup vs baseline: 1.2063x; 1.0157x over previous
"""3-layer GAT (50k nodes, 1.6M edges) on 8 Trainium2 NeuronCores — v3.

v2 edge-parallel-by-destination strategy, with the engine-level hotspots
rebalanced based on the NTFF profile (Vector 85% busy, GpSimd ~100% busy
during aggregation, Scalar 3%):
  - gathers rotate across 4 SWDGE queues (was 1).
  - one-hot build: ONE batched is_equal per chunk for all blocks.
  - pair transposes: 4 per PSUM tile, drained by the Scalar engine.
  - alde matmuls col-sliced into ONE PSUM tile per chunk, one scalar drain.
  - single gather tile per chunk (A+B) -> single zt/lrelu/exp/msg ops.
  - table rows are [h | als] (no ones columns); the per-edge weight lands
    in the message tile via a Scalar-engine copy instead.
  - PSUM->SBUF drains (eacc, eluT, alde, ohT, table rows) moved to the
    Scalar engine; DVE keeps only the arithmetic.
"""

import numpy as np
import ml_dtypes

P = 128
N_NODES = 50000
N_EDGES = 1600000
F_IN = 128
N_CORES = 8
RNODES = N_NODES // N_CORES          # 6250 nodes per core
W = 64                               # scatter window (nodes)
NWIN = (RNODES + W - 1) // W         # 98 windows per core
BCHUNK = (RNODES + P - 1) // P       # 49 table-build chunks per region
HALF = 25000                         # gather table half size (int16 limit)
CHUNK_BLOCKS = 48                    # max 128-edge blocks per aggregation chunk
NQ = 4                               # SWDGE queues for gathers
SPLITW = 88                          # eluT AllGather split point (windows)
SPLITC = SPLITW * W                  # ... in eluT columns (= 44 chunks)

# per-layer (H, C, ELEM): ELEM = bf16 elements per table row (256B aligned)
# row layout: [h (H*C) | als (H) | pad]
LAYERS = [
    dict(H=4, C=16, FIN=128, ELEM=128),
    dict(H=4, C=16, FIN=64, ELEM=128),
    dict(H=6, C=40, FIN=64, ELEM=256),
]

BF16 = ml_dtypes.bfloat16


def _preprocess(edge_index):
    """Sort edges by dst, split per core / window / table-half, pad each
    (window, half) group to a uniform (across cores) multiple of 128."""
    src = np.concatenate([edge_index[0], np.arange(N_NODES, dtype=np.int64)])
    dst = np.concatenate([edge_index[1], np.arange(N_NODES, dtype=np.int64)])
    order = np.argsort(dst, kind="stable")
    src = src[order].astype(np.int64)
    dst = dst[order].astype(np.int64)

    nwh = N_CORES * NWIN
    counts = np.zeros((N_CORES, NWIN, 2), np.int64)
    groups = [[None, None] for _ in range(nwh)]
    for d in range(N_CORES):
        lo_d = d * RNODES
        for w in range(NWIN):
            a = lo_d + w * W
            b = min(lo_d + (w + 1) * W, lo_d + RNODES)
            i0 = np.searchsorted(dst, a)
            i1 = np.searchsorted(dst, b)
            s = src[i0:i1]
            t = dst[i0:i1]
            selA = s < HALF
            groups[d * NWIN + w][0] = (s[selA], t[selA] - a)
            groups[d * NWIN + w][1] = (s[~selA] - HALF, t[~selA] - a)
            counts[d, w, 0] = int(selA.sum())
            counts[d, w, 1] = int((~selA).sum())

    mA = np.ceil(counts[:, :, 0].max(axis=0) / P).astype(int)   # [NWIN]
    mB = np.ceil(counts[:, :, 1].max(axis=0) / P).astype(int)

    chunks = []
    cur, cur_blocks = [], 0
    for w in range(NWIN):
        blk = int(mA[w] + mB[w])
        assert blk <= CHUNK_BLOCKS, f"window {w} too big: {blk} blocks"
        if cur_blocks + blk > CHUNK_BLOCKS:
            chunks.append(cur)
            cur, cur_blocks = [], 0
        cur.append(w)
        cur_blocks += blk
    if cur:
        chunks.append(cur)

    totA = int(mA.sum()) * P
    totB = int(mB.sum()) * P
    nblk_tot = int(mA.sum() + mB.sum())

    meta = dict(mA=mA, mB=mB, chunks=chunks, totA=totA, totB=totB,
                nblk_tot=nblk_tot)

    percore = []
    for d in range(N_CORES):
        eA = np.zeros(totA, np.int16)
        eB = np.zeros(totB, np.int16)
        dstcol = np.full(nblk_tot * P, 99.0, BF16)
        pa = pb = 0
        gb = 0
        for ws in chunks:
            for w in ws:
                s, tl = groups[d * NWIN + w][0]
                n = len(s)
                eA[pa:pa + n] = s
                dstcol[gb * P: gb * P + n] = tl.astype(BF16)
                pa += mA[w] * P
                gb += int(mA[w])
            for w in ws:
                s, tl = groups[d * NWIN + w][1]
                n = len(s)
                eB[pb:pb + n] = s
                dstcol[gb * P: gb * P + n] = tl.astype(BF16)
                pb += mB[w] * P
                gb += int(mB[w])
        idxA = np.tile(eA.reshape(-1, 16).T, (8, 1)) if totA else np.zeros((128, 0), np.int16)
        idxB = np.tile(eB.reshape(-1, 16).T, (8, 1)) if totB else np.zeros((128, 0), np.int16)
        percore.append(dict(
            idxA=np.ascontiguousarray(idxA),
            idxB=np.ascontiguousarray(idxB),
            dstcol=np.ascontiguousarray(dstcol.reshape(nblk_tot, P).T),
        ))
    return meta, percore


def _block_table(meta):
    """Per chunk: block layout [A-blocks by window | B-blocks by window]."""
    mA, mB, chunks = meta["mA"], meta["mB"], meta["chunks"]
    out = []
    gb = 0
    offA = offB = 0
    for ws in chunks:
        nAblk = int(sum(mA[w] for w in ws))
        nBblk = int(sum(mB[w] for w in ws))
        blocks = []   # (w, half, local_block_in_chunk, global_block)
        lb = 0
        for w in ws:
            for _ in range(int(mA[w])):
                blocks.append((w, 0, lb, gb)); lb += 1; gb += 1
        for w in ws:
            for _ in range(int(mB[w])):
                blocks.append((w, 1, lb, gb)); lb += 1; gb += 1
        # pairs within each (window, half) run; lone pair at odd run ends
        pairs = []   # (local_block_of_first, window)
        i = 0
        nblk = len(blocks)
        while i < nblk:
            w_i, half_i = blocks[i][0], blocks[i][1]
            j = i
            while j < nblk and blocks[j][0] == w_i and blocks[j][1] == half_i:
                j += 1
            for k in range(i, j, 2):
                pairs.append((k, w_i))
            i = j
        out.append(dict(ws=ws, nAblk=nAblk, nBblk=nBblk, blocks=blocks,
                        offA=offA, offB=offB, pairs=pairs))
        offA += nAblk * P
        offB += nBblk * P
    return out


def _prep_weights(inputs):
    """Host-side constant prep: combined [W | W@As | W@Ad] per layer."""
    x = np.asarray(inputs["x"], np.float32)
    d = {}
    d["xT"] = np.ascontiguousarray(x.T).astype(BF16)            # [128, 50000]
    for i, (wk, ask, adk, bk) in enumerate(
            [("W1", "a1s", "a1d", "b1"), ("W2", "a2s", "a2d", "b2"),
             ("W3", "a3s", "a3d", "b3")]):
        L = LAYERS[i]
        H, C = L["H"], L["C"]
        Wm = np.asarray(inputs[wk], np.float32)                  # [FIN, H*C]
        a_s = np.asarray(inputs[ask], np.float32)                # [H, C]
        a_d = np.asarray(inputs[adk], np.float32)
        b = np.asarray(inputs[bk], np.float32)
        # als[n,h] = sum_c h[n,h,c]*a_s[h,c] = h @ Asm, Asm[h*C+c, h]=a_s[h,c]
        Asm = np.zeros((H * C, H), np.float32)
        Adm = np.zeros((H * C, H), np.float32)
        for h in range(H):
            Asm[h * C:(h + 1) * C, h] = a_s[h]
            Adm[h * C:(h + 1) * C, h] = a_d[h]
        CW = np.concatenate([Wm, Wm @ Asm, Wm @ Adm], axis=1)    # [FIN, HC+2H]
        d[f"CW{i}"] = CW.astype(BF16)
        d[f"b{i}"] = np.tile(b.reshape(1, -1), (P, 1)).astype(np.float32)
    return d


def build_program(meta, n_cores=None):
    import concourse.bacc as bacc
    import concourse.bass as bass
    import concourse.tile as tile
    import concourse.mybir as mybir
    from concourse.masks import make_identity

    dt = mybir.dt
    AF = mybir.ActivationFunctionType
    OP = mybir.AluOpType

    if n_cores is None:
        n_cores = N_CORES
    mA, mB, chunks = meta["mA"], meta["mB"], meta["chunks"]
    totA, totB, nblk_tot = meta["totA"], meta["totB"], meta["nblk_tot"]
    btab = _block_table(meta)
    max_npair = max(len(cb["pairs"]) for cb in btab)

    nc = bacc.Bacc("TRN2", target_bir_lowering=False, debug=False,
                   num_swdge_queues=NQ)

    # ---------------- I/O ----------------
    t_xT = nc.dram_tensor("xT", [P, N_NODES], dt.bfloat16, kind="ExternalInput")
    t_CW = [nc.dram_tensor(f"CW{i}", [LAYERS[i]["FIN"],
                                      LAYERS[i]["H"] * LAYERS[i]["C"] + 2 * LAYERS[i]["H"]],
                           dt.bfloat16, kind="ExternalInput") for i in range(3)]
    t_b = [nc.dram_tensor(f"b{i}", [P, LAYERS[i]["H"] * LAYERS[i]["C"] if i < 2 else LAYERS[i]["C"]],
                          dt.float32, kind="ExternalInput") for i in range(3)]
    t_sel = nc.dram_tensor("sel", [P, N_CORES], dt.float32, kind="ExternalInput")
    t_idxA = nc.dram_tensor("idxA", [P, max(totA // 16, 1)], dt.int16, kind="ExternalInput")
    t_idxB = nc.dram_tensor("idxB", [P, max(totB // 16, 1)], dt.int16, kind="ExternalInput")
    t_dstcol = nc.dram_tensor("dstcol", [P, nblk_tot], dt.bfloat16, kind="ExternalInput")
    t_out = nc.dram_tensor("out", [RNODES, LAYERS[2]["C"]], dt.float32, kind="ExternalOutput")

    # ---------------- internal DRAM ----------------
    t_tab12 = nc.dram_tensor("tab12", [N_NODES, 128], dt.bfloat16)
    t_tab3 = nc.dram_tensor("tab3", [N_NODES, 256], dt.bfloat16)
    # eluT AllGather split into two contiguous buffer pairs. BOTH
    # collectives are emitted after the layer's gathers (a mid-loop
    # collective stalls the gather stream while cores sync — measured
    # +0.6ms), but the first one's input DMA fires mid-loop, so it
    # executes right after the last gather retires, before the trailing
    # epilogues finish — letting most of the next table build start early.
    NCOLS2 = BCHUNK * P - SPLITC
    t_ccinA = [nc.dram_tensor(f"ccinA{i}", [64, SPLITC], dt.bfloat16) for i in range(2)]
    t_ccinB = [nc.dram_tensor(f"ccinB{i}", [64, NCOLS2], dt.bfloat16) for i in range(2)]
    t_agA = [nc.dram_tensor(f"agA{i}", [N_CORES, 64, SPLITC], dt.bfloat16,
                            addr_space="Shared") for i in range(2)]
    t_agB = [nc.dram_tensor(f"agB{i}", [N_CORES, 64, NCOLS2], dt.bfloat16,
                            addr_space="Shared") for i in range(2)]

    core_ids = list(range(n_cores))
    qc = [0]   # gather queue rotation counter

    with tile.TileContext(nc) as tc:
        with tc.tile_pool(name="const", bufs=1) as cpool, \
             tc.tile_pool(name="persist", bufs=1) as ppool, \
             tc.tile_pool(name="build", bufs=3) as bpool, \
             tc.tile_pool(name="gath", bufs=2) as gpool, \
             tc.tile_pool(name="edge", bufs=2) as epool, \
             tc.tile_pool(name="blk", bufs=6) as kpool, \
             tc.tile_pool(name="win", bufs=2) as wpool, \
             tc.tile_pool(name="bps", bufs=2, space="PSUM") as bps, \
             tc.tile_pool(name="aps", bufs=2, space="PSUM") as aps:

            # ----- constants -----
            ident = cpool.tile([P, P], dt.bfloat16, tag="ident")
            make_identity(nc, ident[:])
            iota_i = cpool.tile([P, W], dt.int32, tag="iotai")
            nc.gpsimd.iota(iota_i[:], [[1, W]], base=0, channel_multiplier=0)
            iota_row = cpool.tile([P, W], dt.bfloat16, tag="iotarow")
            nc.vector.tensor_copy(iota_row[:], iota_i[:])
            sel = cpool.tile([P, N_CORES], dt.float32, tag="sel")
            nc.sync.dma_start(sel[:], t_sel[:])
            CW_sb, b_sb = [], []
            for i in range(3):
                L = LAYERS[i]
                HC2H = L["H"] * L["C"] + 2 * L["H"]
                wt = cpool.tile([L["FIN"], HC2H], dt.bfloat16, tag=f"CW{i}")
                nc.sync.dma_start(wt[:], t_CW[i][:])
                CW_sb.append(wt)
                bt = cpool.tile([P, L["H"] * L["C"] if i < 2 else L["C"]], dt.float32, tag=f"b{i}")
                nc.sync.dma_start(bt[:], t_b[i][:])
                b_sb.append(bt)

            # ----- resident edge data -----
            idxA_sb = ppool.tile([P, max(totA // 16, 1)], dt.int16, tag="idxA")
            nc.sync.dma_start(idxA_sb[:], t_idxA[:])
            idxB_sb = ppool.tile([P, max(totB // 16, 1)], dt.int16, tag="idxB")
            nc.sync.dma_start(idxB_sb[:], t_idxB[:])
            dstcol_sb = ppool.tile([P, nblk_tot], dt.bfloat16, tag="dstcol")
            nc.sync.dma_start(dstcol_sb[:], t_dstcol[:])

            # persistent row buffers for the table build (pad stays zero)
            GB = 7   # build chunks per DMA group (49 = 7 even groups)
            rowbufs = []
            for i in range(3):
                rb = ppool.tile([P, GB, 256], dt.bfloat16, tag=f"rowb{i}",
                                name=f"rowb{i}")
                nc.vector.memset(rb[:], 0.0)
                rowbufs.append(rb)

            def build_table(li):
                """Build gather table for layer li; returns own-region al_d."""
                L = LAYERS[li]
                H, C, FIN, ELEM = L["H"], L["C"], L["FIN"], L["ELEM"]
                HC = H * C
                ROW = HC + H
                t_tab = t_tab12 if li < 2 else t_tab3
                aldmy = ppool.tile([P, BCHUNK, H], dt.bfloat16, tag="aldmy",
                                   name="aldmy")
                nc.vector.memset(aldmy[:], 0.0)
                it = 0
                for r in range(N_CORES):
                    for c0 in range(0, BCHUNK, GB):
                        gn = min(GB, BCHUNK - c0)
                        n0 = r * RNODES + c0 * P
                        ntot = min(gn * P, RNODES - c0 * P)
                        lhsT = bpool.tile([FIN, GB * P], dt.bfloat16, tag="lhsT")
                        if li == 0:
                            nc.scalar.dma_start(lhsT[:, :ntot],
                                                t_xT[:, n0:n0 + ntot])
                        else:
                            lo, hi = c0 * P, c0 * P + ntot
                            if hi <= SPLITC:
                                nc.scalar.dma_start(
                                    lhsT[:, :ntot],
                                    t_agA[li - 1][r, :, lo:hi])
                            elif lo >= SPLITC:
                                nc.scalar.dma_start(
                                    lhsT[:, :ntot],
                                    t_agB[li - 1][r, :, lo - SPLITC:hi - SPLITC])
                            else:
                                nc.scalar.dma_start(
                                    lhsT[:, :SPLITC - lo],
                                    t_agA[li - 1][r, :, lo:SPLITC])
                                nc.scalar.dma_start(
                                    lhsT[:, SPLITC - lo:ntot],
                                    t_agB[li - 1][r, :, 0:hi - SPLITC])
                        rb = rowbufs[it % 3]
                        it += 1
                        for k in range(gn):
                            c = c0 + k
                            cols = min(P, RNODES - c * P)
                            hps = bps.tile([P, HC + 2 * H], dt.float32,
                                           space="PSUM", tag="hps", bufs=2)
                            nc.tensor.matmul(hps[:cols, :],
                                             lhsT[:, k * P:k * P + cols],
                                             CW_sb[li][:], start=True, stop=True)
                            # own-region al_d accumulate (sel mask) from PSUM
                            nc.vector.scalar_tensor_tensor(
                                out=aldmy[:cols, c, :],
                                in0=hps[:cols, HC + H:HC + 2 * H],
                                scalar=sel[:cols, r:r + 1],
                                in1=aldmy[:cols, c, :],
                                op0=OP.mult, op1=OP.add)
                            # table row [h | als] via scalar-engine PSUM drain
                            nc.scalar.activation(rb[:cols, k, 0:ROW],
                                                 hps[:cols, 0:ROW], AF.Copy)
                        if ntot % P == 0:
                            nc.sync.dma_start(
                                t_tab[n0:n0 + ntot, 0:ELEM].rearrange(
                                    "(n p) c -> p n c", p=P),
                                rb[:, 0:gn, 0:ELEM])
                        else:
                            for k in range(gn):
                                c = c0 + k
                                cols = min(P, RNODES - c * P)
                                nc.sync.dma_start(
                                    t_tab[r * RNODES + c * P:
                                          r * RNODES + c * P + cols, 0:ELEM],
                                    rb[:cols, k, 0:ELEM])
                return aldmy

            def aggregate(li, aldmy_sb, aldsw_sb):
                L = LAYERS[li]
                H, C, ELEM = L["H"], L["C"], L["ELEM"]
                HC = H * C
                ROW = HC + H
                RW = H + HC   # msg width: [w | w*h]
                t_tab = t_tab12 if li < 2 else t_tab3
                eluT = (ppool.tile([64, BCHUNK * P], dt.bfloat16, tag="eluT",
                                   name="eluT") if li < 2 else None)

                # --- per-window ald pair table, built once per layer ---
                # apw[:, w, :] = [ald_w(slots) on parts 0:64 in cols 0:H |
                #                 ald_w(slots) on parts 64:128 in cols H:2H]
                apw = ppool.tile([P, NWIN, 2 * H], dt.bfloat16, tag="apw",
                                 name="apw")
                apw_v = apw[:].rearrange("p (n t) h -> p t n h", t=2)
                nc.vector.memset(apw[0:W, :, H:2 * H], 0.0)
                nc.vector.memset(apw[W:P, :, 0:H], 0.0)
                nc.vector.tensor_copy(apw_v[0:W, 0, :, 0:H], aldmy_sb[0:W, :, :])
                nc.vector.tensor_copy(apw_v[0:W, 1, :, 0:H], aldsw_sb[0:W, :, :])
                nc.vector.tensor_copy(apw_v[W:P, 0, :, H:2 * H], aldsw_sb[W:P, :, :])
                nc.vector.tensor_copy(apw_v[W:P, 1, :, H:2 * H], aldmy_sb[W:P, :, :])

                for ci, cb in enumerate(btab):
                    ws, nAblk, nBblk = cb["ws"], cb["nAblk"], cb["nBblk"]
                    nblk = nAblk + nBblk
                    gb0 = cb["blocks"][0][3]

                    # ---- gathers: one tile, A blocks then B blocks ----
                    # (the SWDGE gather ucode rejects num_idxs > 1024: GMAX
                    # 12 and 16 both fault at runtime — keep 8 blocks max)
                    GMAX = 8
                    g = gpool.tile([P, CHUNK_BLOCKS, ELEM], dt.bfloat16, tag="g")
                    if nAblk:
                        for ob in range(0, nAblk, GMAX):
                            nb = min(GMAX, nAblk - ob)
                            o0 = cb["offA"] + ob * P
                            nc.gpsimd.dma_gather(
                                g[:, ob:ob + nb, :], t_tab[0:HALF, 0:ELEM],
                                idxA_sb[:, o0 // 16:(o0 + nb * P) // 16],
                                nb * P, nb * P, ELEM, queue_num=qc[0] % NQ)
                            qc[0] += 1
                    if nBblk:
                        for ob in range(0, nBblk, GMAX):
                            nb = min(GMAX, nBblk - ob)
                            o0 = cb["offB"] + ob * P
                            nc.gpsimd.dma_gather(
                                g[:, nAblk + ob:nAblk + ob + nb, :],
                                t_tab[HALF:N_NODES, 0:ELEM],
                                idxB_sb[:, o0 // 16:(o0 + nb * P) // 16],
                                nb * P, nb * P, ELEM, queue_num=qc[0] % NQ)
                            qc[0] += 1

                    # ---- batched one-hot build: one is_equal per chunk ----
                    oh_all = epool.tile([P, CHUNK_BLOCKS + 1, W],
                                        dt.bfloat16, tag="oh")
                    nc.vector.tensor_tensor(
                        out=oh_all[:, 0:nblk, :],
                        in0=dstcol_sb[:, gb0:gb0 + nblk].unsqueeze(-1)
                            .to_broadcast([P, nblk, W]),
                        in1=iota_row[:].unsqueeze(1).to_broadcast([P, nblk, W]),
                        op=OP.is_equal)
                    nc.vector.memset(oh_all[:, nblk, :], 0.0)

                    # ---- pairs within each (window, half) run ----
                    # lone pair at a run end: its second alde half spills into
                    # the next block's column and is overwritten by that
                    # block's own pair (in-order PE).
                    pairs = cb["pairs"]
                    npair = len(pairs)

                    # ---- pair transposes, 4 per PSUM tile, scalar drain ----
                    ohT_all = kpool.tile([P, max_npair, P],
                                         dt.bfloat16, tag="ohT", bufs=2)
                    for t0 in range(0, npair, 4):
                        nk = min(4, npair - t0)
                        tp = aps.tile([P, 512], dt.bfloat16, space="PSUM",
                                      tag="tp", bufs=2)
                        for k in range(nk):
                            lb0 = pairs[t0 + k][0]
                            nc.tensor.transpose(
                                tp[:, k * P:(k + 1) * P],
                                oh_all[:, lb0:lb0 + 2, :].rearrange(
                                    "p a b -> p (a b)"),
                                ident[:])
                        nc.scalar.activation(
                            ohT_all[:, t0:t0 + nk, :].rearrange("p n e -> p (n e)"),
                            tp[:, 0:nk * P], AF.Copy)

                    # ---- alde matmuls, block-aligned cols in one PSUM tile ----
                    aldeps = aps.tile([P, (CHUNK_BLOCKS + 2) * H],
                                      dt.float32, space="PSUM", tag="aldeps",
                                      bufs=2)
                    for pi, (lb0, w_) in enumerate(pairs):
                        nc.tensor.matmul(aldeps[:, lb0 * H:(lb0 + 2) * H],
                                         ohT_all[:, pi, :], apw[:, w_, :],
                                         start=True, stop=True)
                    alde_sb = epool.tile([P, (CHUNK_BLOCKS + 2) * H],
                                         dt.float32, tag="alde")
                    nc.scalar.activation(alde_sb[:, 0:nblk * H],
                                         aldeps[:, 0:nblk * H], AF.Copy)
                    alde_v = alde_sb[:].rearrange("p (n h) -> p n h", h=H)

                    # z = al_s + al_d ; lrelu ; w = exp  (batched per chunk)
                    zt = epool.tile([P, CHUNK_BLOCKS, H], dt.float32, tag="zt")
                    nc.vector.tensor_tensor(
                        out=zt[:, 0:nblk, :],
                        in0=g[:, 0:nblk, HC:HC + H],
                        in1=alde_v[:, 0:nblk, :], op=OP.add)
                    nc.vector.scalar_tensor_tensor(
                        out=zt[:, 0:nblk, :], in0=zt[:, 0:nblk, :], scalar=0.2,
                        in1=zt[:, 0:nblk, :], op0=OP.mult, op1=OP.max)
                    wa = epool.tile([P, CHUNK_BLOCKS, H], dt.bfloat16, tag="wa")
                    nc.scalar.activation(wa[:, 0:nblk, :], zt[:, 0:nblk, :],
                                         AF.Exp)

                    # ---- messages: m = [wa | h * wa] ----
                    m_t = epool.tile([P, CHUNK_BLOCKS, RW], dt.bfloat16, tag="m")
                    nc.scalar.activation(m_t[:, 0:nblk, 0:H], wa[:, 0:nblk, :],
                                         AF.Copy)
                    nc.vector.tensor_tensor(
                        out=m_t[:, 0:nblk, H:RW].rearrange(
                            "p n (h c) -> p n h c", c=C),
                        in0=g[:, 0:nblk, 0:HC].rearrange(
                            "p n (h c) -> p n h c", c=C),
                        in1=wa[:, 0:nblk, :].unsqueeze(-1).to_broadcast(
                            [P, nblk, H, C]),
                        op=OP.mult)

                    # ---- scatter per window ----
                    perwin = {}
                    for (w, half, lb, gbk) in cb["blocks"]:
                        perwin.setdefault(w, []).append(lb)
                    nw = len(ws)
                    eacc = wpool.tile([W, nw, RW], dt.float32, tag="eacc")
                    for wi, w in enumerate(ws):
                        blks = perwin[w]
                        acc = aps.tile([W, RW], dt.float32, space="PSUM",
                                       tag="acc", bufs=2)
                        for j, lb in enumerate(blks):
                            nc.tensor.matmul(acc[:], oh_all[:, lb, :],
                                             m_t[:, lb, :],
                                             start=(j == 0), stop=(j == len(blks) - 1))
                        wn = min(W, RNODES - w * W)
                        nc.scalar.activation(eacc[:wn, wi, :], acc[:wn, :],
                                             AF.Copy)

                    # ---- batched epilogue over the chunk's windows ----
                    rs = wpool.tile([W, nw, H], dt.float32, tag="rs")
                    nc.vector.reciprocal(rs[:], eacc[:, :, 0:H])
                    on = wpool.tile([W, nw, HC], dt.float32, tag="on")
                    nc.vector.tensor_tensor(
                        out=on[:].rearrange("p n (h c) -> p n h c", c=C),
                        in0=eacc[:, :, H:RW].rearrange("p n (h c) -> p n h c", c=C),
                        in1=rs[:].unsqueeze(-1).to_broadcast([W, nw, H, C]),
                        op=OP.mult)
                    if li < 2:
                        nc.vector.tensor_tensor(
                            out=on[:],
                            in0=on[:],
                            in1=b_sb[li][0:W, :].unsqueeze(1).to_broadcast([W, nw, HC]),
                            op=OP.add)
                        # elu = exp(min(x,0)) + max(x,0) - 1
                        # min(x,0) = -relu(-x); both steps on the scalar engine
                        zm = wpool.tile([W, nw, HC], dt.float32, tag="zm")
                        nc.scalar.activation(zm[:], on[:], AF.Relu, scale=-1.0)
                        ez = wpool.tile([W, nw, HC], dt.float32, tag="ez")
                        nc.scalar.activation(ez[:], zm[:], AF.Exp, scale=-1.0)
                        elf = wpool.tile([W, nw, HC], dt.float32, tag="elf")
                        nc.vector.scalar_tensor_tensor(
                            out=elf[:], in0=on[:], scalar=0.0,
                            in1=ez[:], op0=OP.max, op1=OP.add)
                        el = wpool.tile([W, nw, HC], dt.bfloat16, tag="el")
                        nc.scalar.activation(el[:], elf[:], AF.Copy, bias=-1.0)
                        # transpose each window -> eluT slice (scalar drain)
                        for wi, w in enumerate(ws):
                            wn = min(W, RNODES - w * W)
                            tps = aps.tile([P, 512], dt.bfloat16, space="PSUM",
                                           tag="tp", bufs=2)
                            nc.tensor.transpose(tps[:HC, :wn], el[:wn, wi, :],
                                                ident[:wn, :wn])
                            nc.scalar.activation(
                                eluT[:, w * W:w * W + wn], tps[:HC, :wn],
                                AF.Copy)
                        # stage the first AllGather's input as soon as
                        # windows < SPLITW are transposed (sync queue only)
                        if ws[-1] >= SPLITW - 1 and (ci == 0 or
                                                     btab[ci - 1]["ws"][-1] < SPLITW - 1):
                            nc.sync.dma_start(t_ccinA[li][:, :],
                                              eluT[:, 0:SPLITC])
                    else:
                        # mean over heads -> [*, nw, C]; 1/H folded into the
                        # head-sum via a scalar-engine scaled copy of on
                        ons = wpool.tile([W, nw, HC], dt.float32, tag="ons")
                        nc.scalar.activation(ons[:], on[:], AF.Copy,
                                             scale=1.0 / H)
                        mn = wpool.tile([W, nw, C], dt.float32, tag="mn")
                        nc.vector.tensor_reduce(
                            mn[:], ons[:].rearrange("p n (h c) -> p n c h", h=H),
                            axis=mybir.AxisListType.X, op=OP.add)
                        nc.vector.tensor_tensor(
                            out=mn[:], in0=mn[:],
                            in1=b_sb[2][0:W, :].unsqueeze(1).to_broadcast([W, nw, C]),
                            op=OP.add)
                        # elu sans the -1 (log_softmax is shift-invariant)
                        zm = wpool.tile([W, nw, C], dt.float32, tag="zm3")
                        nc.scalar.activation(zm[:], mn[:], AF.Relu, scale=-1.0)
                        ez = wpool.tile([W, nw, C], dt.float32, tag="ez3")
                        nc.scalar.activation(ez[:], zm[:], AF.Exp, scale=-1.0)
                        el = wpool.tile([W, nw, C], dt.float32, tag="el3")
                        nc.vector.scalar_tensor_tensor(
                            out=el[:], in0=mn[:], scalar=0.0,
                            in1=ez[:], op0=OP.max, op1=OP.add)
                        # log_softmax (batched)
                        mx = wpool.tile([W, nw, 1], dt.float32, tag="mx")
                        nc.vector.tensor_reduce(mx[:], el[:],
                                                axis=mybir.AxisListType.X, op=OP.max)
                        xm = wpool.tile([W, nw, C], dt.float32, tag="xm")
                        nc.vector.tensor_tensor(
                            out=xm[:], in0=el[:],
                            in1=mx[:].to_broadcast([W, nw, C]), op=OP.subtract)
                        ex = wpool.tile([W, nw, C], dt.float32, tag="ex3")
                        nc.scalar.activation(ex[:], xm[:], AF.Exp)
                        sm = wpool.tile([W, nw, 1], dt.float32, tag="sm")
                        nc.vector.tensor_reduce(sm[:], ex[:],
                                                axis=mybir.AxisListType.X, op=OP.add)
                        ls = wpool.tile([W, nw, 1], dt.float32, tag="ls")
                        nc.scalar.activation(ls[:], sm[:], AF.Ln)
                        fo = wpool.tile([W, nw, C], dt.float32, tag="fo")
                        nc.vector.tensor_tensor(
                            out=fo[:], in0=xm[:],
                            in1=ls[:].to_broadcast([W, nw, C]), op=OP.subtract)
                        # write out all full windows of the chunk
                        w0 = ws[0]
                        if ws[-1] * W + W <= RNODES:
                            nc.sync.dma_start(
                                t_out[w0 * W:ws[-1] * W + W, :].rearrange(
                                    "(n p) c -> p n c", p=W),
                                fo[:, :, :])
                        else:
                            for wi, w in enumerate(ws):
                                wn = min(W, RNODES - w * W)
                                nc.sync.dma_start(t_out[w * W:w * W + wn, :],
                                                  fo[:wn, wi, :])
                if li < 2:
                    nc.gpsimd.collective_compute(
                        "AllGather", mybir.AluOpType.bypass,
                        replica_groups=[core_ids],
                        ins=[t_ccinA[li][:, :]],
                        outs=[t_agA[li][:, :, :].rearrange("r p n -> (r p) n")],
                    )
                    nc.sync.dma_start(t_ccinB[li][:, :], eluT[:, SPLITC:])
                    nc.gpsimd.collective_compute(
                        "AllGather", mybir.AluOpType.bypass,
                        replica_groups=[core_ids],
                        ins=[t_ccinB[li][:, :]],
                        outs=[t_agB[li][:, :, :].rearrange("r p n -> (r p) n")],
                    )

            for li in range(3):
                aldmy_sb = build_table(li)
                H_li = LAYERS[li]["H"]
                aldsw_sb = ppool.tile([P, BCHUNK, H_li], dt.bfloat16,
                                      tag="aldsw", name="aldsw")
                nc.sync.dma_start(aldsw_sb[0:64, :, :], aldmy_sb[64:128, :, :])
                nc.sync.dma_start(aldsw_sb[64:128, :, :], aldmy_sb[0:64, :, :])
                aggregate(li, aldmy_sb, aldsw_sb)

    nc.compile()
    return nc


def prepare(inputs):
    meta, percore = _preprocess(np.asarray(inputs["edge_index"]))
    wd = _prep_weights(inputs)
    nc = build_program(meta)

    in_maps = []
    for d in range(N_CORES):
        sel = np.zeros((P, N_CORES), np.float32)
        sel[:, d] = 1.0
        m = dict(
            xT=wd["xT"],
            sel=sel,
            idxA=percore[d]["idxA"], idxB=percore[d]["idxB"],
            dstcol=percore[d]["dstcol"],
        )
        for i in range(3):
            m[f"CW{i}"] = wd[f"CW{i}"]
            m[f"b{i}"] = wd[f"b{i}"]
        in_maps.append(m)
    return nc, in_maps


def kernel(x, edge_index, W1, a1s, a1d, b1, W2, a2s, a2d, b2, W3, a3s, a3d, b3):
    from concourse.bass_utils import run_bass_kernel_spmd

    inputs = dict(x=x, edge_index=edge_index, W1=W1, a1s=a1s, a1d=a1d, b1=b1,
                  W2=W2, a2s=a2s, a2d=a2d, b2=b2, W3=W3, a3s=a3s, a3d=a3d, b3=b3)
    nc, in_maps = prepare(inputs)
    res = run_bass_kernel_spmd(nc, in_maps, core_ids=list(range(N_CORES)))
    out = np.concatenate([res.results[d]["out"] for d in range(N_CORES)], axis=0)
    return out.astype(np.float32)
